# revision 5
# baseline (speedup 1.0000x reference)
"""Trainium2 Bass kernel for EvolveGCN-O forward (GCN message passing).

Math (reference):
    h   = x @ Wp + bp
    W   = LSTM-evolved weight from initial_weight (one step, h0=c0=IW)
    hw  = h @ W
    out = D^-1/2 (A+I) D^-1/2 hw + b_gcn

Factored for the kernel:
    out[d] = dinv[d] * (sum_{e: dst=d} dinv[src_e] * x[src_e]) @ (Wp @ W)
             + s2[d]*dinv[d]*(bp @ W) + b_gcn
with s2[d] = sum_{e in(d)} dinv[src_e] (self loops included as edges).

Distribution: nodes (dsts) sharded over 8 NeuronCores (serpentine by degree);
the dinv-scaled fp16 source-feature table is replicated in every core's SBUF;
each core gathers its edges' source rows via SWDGE prepare-only descriptor
generation + trigger_dma on 4 queues (the prep never blocks the GpSimd engine
on transfer completion), performs the segment sum on the TensorEngine via
one-hot masks (built on GpSimd/Vector), then applies the (tiny, replicated)
evolved-weight matmul.
"""

import numpy as np

N_NODES = 10000
N_EDGES = 320000
IN_DIM = 128
HID = 256
M = 8                    # NeuronCores
NP = 10240               # padded node count (mult of 128)
RANKS = NP // 128        # 80
NPC = NP // M            # 1280 padded dsts per core
NGRP = NPC // 128        # 10 dst blocks of 128 per core
DUMMY_DL = 999.0         # dst_local value that matches no column

_cache = {}


def _round_up(x, m):
    return (x + m - 1) // m * m


def _build_module(ni_list):
    """Build+compile the Bacc module for given per-group edge counts."""
    import concourse.bacc as bacc
    import concourse.mybir as mybir
    import concourse.tile as tile

    TOT = int(sum(ni_list))
    nc = bacc.Bacc("TRN2", target_bir_lowering=False, debug=False,
                   num_devices=M, num_swdge_queues=4)
    f32, f16, i16 = mybir.dt.float32, mybir.dt.float16, mybir.dt.int16

    # ---- DRAM inputs ----
    xs_in = nc.dram_tensor("xs_tiled", [128, RANKS * 128], f16, kind="ExternalInput").ap()
    idx_in = nc.dram_tensor("idx", [128, TOT // 16], i16, kind="ExternalInput").ap()
    dl_in = nc.dram_tensor("dl", [128, TOT // 128], f16, kind="ExternalInput").ap()
    iota_in = nc.dram_tensor("iota", [128, 128], f16, kind="ExternalInput").ap()
    ws_in = nc.dram_tensor("WsumT", [256, 1024], f32, kind="ExternalInput").ap()
    iw_in = nc.dram_tensor("IW", [256, 256], f32, kind="ExternalInput").ap()
    iwt_in = nc.dram_tensor("IWT", [256, 256], f32, kind="ExternalInput").ap()
    wpt_in = nc.dram_tensor("WpT", [256, 128], f32, kind="ExternalInput").ap()
    bsum_in = nc.dram_tensor("bsum", [1, 1024], f32, kind="ExternalInput").ap()
    bp_in = nc.dram_tensor("bp_col", [256, 1], f32, kind="ExternalInput").ap()
    bgcn_in = nc.dram_tensor("b_gcn", [1, 256], f32, kind="ExternalInput").ap()
    ones_in = nc.dram_tensor("ones_row", [1, 128], f32, kind="ExternalInput").ap()
    s2_in = nc.dram_tensor("s2_row", [1, NPC], f32, kind="ExternalInput").ap()
    dri_in = nc.dram_tensor("dri_row", [1, NPC], f32, kind="ExternalInput").ap()
    dcol_in = nc.dram_tensor("dinv_col", [128, NGRP], f32, kind="ExternalInput").ap()

    out_t = nc.dram_tensor("out", [NPC, HID], f32, kind="ExternalOutput").ap()

    def gather_prep(out_ap, in_ap, idxs_ap, num_idxs, queue_num):
        """Prepare-only SWDGE gather (descriptors written, DMA deferred to
        trigger_dma). Tile attributes the deferred SBUF write to this
        instruction's DMASW tick, so consumers wait on DMA completion."""
        eng = nc.gpsimd
        in_ap = in_ap.bitcast(out_ap.dtype) if in_ap.dtype != out_ap.dtype else in_ap
        inst = eng.add_instruction(
            mybir.InstDMAGatherAnt(
                name=f"I-{nc.next_id()}",
                ins=[eng.lower_ap(in_ap), eng.lower_ap(idxs_ap),
                     eng.lower_val_access(eng.to_reg(num_idxs))],
                outs=[eng.lower_ap(out_ap)],
                transpose=False,
                num_idxs=num_idxs,
                elem_size=128,
                stride_bytes_256=0,
                gen_mode=1,
                single_packet=False,
                queue_num=queue_num,
                sbuf_tokens_per_rank=128,
                sbuf_free_dim_per_rank=256,
                sbuf_free_dim_pad_per_rank=0,
                sbuf_byte_offset=0,
            )
        )
        eng._track_prepare_only(inst, queue_num)
        return inst

    with tile.TileContext(nc) as tc:
        with (
            tc.tile_pool(name="stage", bufs=1) as stpool,
            tc.tile_pool(name="persist", bufs=1) as pp,
            tc.tile_pool(name="sp", bufs=3) as spool,
            tc.tile_pool(name="op", bufs=3) as opool,
            tc.tile_pool(name="psg", bufs=2, space="PSUM") as psg,
            tc.tile_pool(name="psl", bufs=1, space="PSUM") as psl,
        ):
            # ---------- phase 0: table + index loads ----------
            xs_sb = pp.tile([128, RANKS * 128], f16)
            idxs = pp.tile([128, TOT // 16], i16)
            dls = pp.tile([128, TOT // 128], f16)
            iota = pp.tile([128, 128], f16)
            HLF = RANKS * 64
            nc.sync.dma_start(out=xs_sb[:, :HLF], in_=xs_in[:, :HLF])
            nc.scalar.dma_start(out=idxs[:], in_=idx_in[:])
            nc.scalar.dma_start(out=xs_sb[:, HLF:], in_=xs_in[:, HLF:])
            nc.sync.dma_start(out=dls[:], in_=dl_in[:])
            nc.sync.dma_start(out=iota[:], in_=iota_in[:])

            # ---------- gathers: prep + trigger everything up front ----------
            arena = pp.tile([128, TOT // 128, 128], f16)
            goff = [0]
            for ni in ni_list:
                goff.append(goff[-1] + int(ni))
            for g in range(NGRP):
                ni = int(ni_list[g])
                nch = ni // 128
                base = goff[g]
                h1 = (nch // 2) * 128
                parts = [(0, h1), (h1, ni)] if h1 else [(0, ni)]
                for k, (lo, hi) in enumerate(parts):
                    if hi <= lo:
                        continue
                    q = (2 * g + k) % 4
                    gather_prep(
                        arena[:, (base + lo) // 128:(base + hi) // 128, :],
                        xs_sb[:],
                        idxs[:, (base + lo) // 16:(base + hi) // 16],
                        hi - lo,
                        q,
                    )
                    nc.gpsimd.trigger_dma(count=None, queue_num=q)

            # ---------- small-tensor loads + LSTM weight evolution ----------
            wsum = pp.tile([128, 2, 1024], f32)
            iw = pp.tile([128, 2, 256], f32)
            iwt = pp.tile([128, 2, 256], f32)
            wpt = pp.tile([128, 2, 128], f32)
            bsum = pp.tile([1, 1024], f32)
            bp_c = pp.tile([128, 2, 1], f32)
            bgcn = pp.tile([1, 256], f32)
            ones = pp.tile([1, 128], f32)
            s2r = pp.tile([1, NPC], f32)
            drir = pp.tile([1, NPC], f32)
            dcol = pp.tile([128, NGRP], f32)
            for t_, s_ in ((wsum, ws_in), (iw, iw_in), (iwt, iwt_in),
                           (wpt, wpt_in), (bp_c, bp_in)):
                nc.scalar.dma_start(out=t_[:], in_=s_.rearrange("(k p) c -> p k c", p=128))
            for t_, s_ in ((bsum, bsum_in), (bgcn, bgcn_in), (ones, ones_in),
                           (s2r, s2_in), (dri_in_t := drir, dri_in),
                           (dcol, dcol_in)):
                nc.sync.dma_start(out=t_[:], in_=s_[:])

            w_ev = pp.tile([128, 2, 256], f32)   # evolved GCN weight W
            Sig = mybir.ActivationFunctionType.Sigmoid
            Tanh = mybir.ActivationFunctionType.Tanh
            for ic in range(2):
                gpsum = psl.tile([128, 1024], f32, space="PSUM", tag="gates")
                for h in range(2):
                    gs = slice(512 * h, 512 * (h + 1))
                    nc.tensor.matmul(out=gpsum[:, gs], lhsT=ones[:, :],
                                     rhs=bsum[:, gs], start=True, stop=False)
                    nc.tensor.matmul(out=gpsum[:, gs],
                                     lhsT=iwt[:, 0, 128 * ic:128 * (ic + 1)],
                                     rhs=wsum[:, 0, gs], start=False, stop=False)
                    nc.tensor.matmul(out=gpsum[:, gs],
                                     lhsT=iwt[:, 1, 128 * ic:128 * (ic + 1)],
                                     rhs=wsum[:, 1, gs], start=False, stop=True)
                si = stpool.tile([128, 256], f32, tag="si")
                sf = stpool.tile([128, 256], f32, tag="sf")
                tg = stpool.tile([128, 256], f32, tag="tg")
                so = stpool.tile([128, 256], f32, tag="so")
                nc.scalar.activation(out=si[:], in_=gpsum[:, 0:256], func=Sig)
                nc.scalar.activation(out=sf[:], in_=gpsum[:, 256:512], func=Sig)
                nc.scalar.activation(out=tg[:], in_=gpsum[:, 512:768], func=Tanh)
                nc.scalar.activation(out=so[:], in_=gpsum[:, 768:1024], func=Sig)
                c1 = stpool.tile([128, 256], f32, tag="c1")
                nc.vector.tensor_tensor(out=c1[:], in0=sf[:], in1=iw[:, ic, :],
                                        op=mybir.AluOpType.mult)
                c2 = stpool.tile([128, 256], f32, tag="c2")
                nc.vector.tensor_tensor(out=c2[:], in0=si[:], in1=tg[:],
                                        op=mybir.AluOpType.mult)
                cc = stpool.tile([128, 256], f32, tag="cc")
                nc.vector.tensor_tensor(out=cc[:], in0=c1[:], in1=c2[:],
                                        op=mybir.AluOpType.add)
                tcc = stpool.tile([128, 256], f32, tag="tcc")
                nc.scalar.activation(out=tcc[:], in_=cc[:], func=Tanh)
                nc.vector.tensor_tensor(out=w_ev[:, ic, :], in0=so[:], in1=tcc[:],
                                        op=mybir.AluOpType.mult)

            wpw = pp.tile([128, 256], f32)       # Wp @ W
            wp_ps = psl.tile([128, 256], f32, space="PSUM", tag="wpw")
            nc.tensor.matmul(out=wp_ps[:], lhsT=wpt[:, 0, :], rhs=w_ev[:, 0, :],
                             start=True, stop=False)
            nc.tensor.matmul(out=wp_ps[:], lhsT=wpt[:, 1, :], rhs=w_ev[:, 1, :],
                             start=False, stop=True)
            nc.vector.tensor_copy(out=wpw[:], in_=wp_ps[:])
            bpw = pp.tile([1, 256], f32)         # bp @ W
            bp_ps = psl.tile([1, 256], f32, space="PSUM", tag="bpw")
            nc.tensor.matmul(out=bp_ps[:], lhsT=bp_c[:, 0, :], rhs=w_ev[:, 0, :],
                             start=True, stop=False)
            nc.tensor.matmul(out=bp_ps[:], lhsT=bp_c[:, 1, :], rhs=w_ev[:, 1, :],
                             start=False, stop=True)
            nc.vector.tensor_copy(out=bpw[:], in_=bp_ps[:])

            # ---------- main: one-hot masks + PE segment sum + output ----------
            xaggT = pp.tile([128, NPC], f32)
            for g in range(NGRP):
                ni = int(ni_list[g])
                nch = ni // 128
                base = goff[g]
                coff = base // 128
                # one-hot mask: st[p, c, d] = (dl[p, c] == d)
                st = spool.tile([128, nch, 128], f16, tag="st")
                nc.vector.tensor_tensor(
                    out=st[:],
                    in0=dls[:, coff:coff + nch]
                        .rearrange("p (c o) -> p c o", o=1)
                        .to_broadcast([128, nch, 128]),
                    in1=iota[:].rearrange("p (o d) -> p o d", o=1)
                        .to_broadcast([128, nch, 128]),
                    op=mybir.AluOpType.is_equal,
                )
                gps = psg.tile([128, 128], f32, space="PSUM", tag="agg")
                for c in range(nch):
                    nc.tensor.matmul(out=gps[:], lhsT=arena[:, coff + c, :],
                                     rhs=st[:, c, :],
                                     start=(c == 0), stop=(c == nch - 1))
                nc.scalar.activation(out=xaggT[:, g * 128:(g + 1) * 128],
                                     in_=gps[:],
                                     func=mybir.ActivationFunctionType.Copy)

                # final: out rows = dinv*(xagg@WpW + s2*bpW + dri*bgcn)
                ops = psg.tile([128, HID], f32, space="PSUM", tag="ops")
                ds = slice(128 * g, 128 * (g + 1))
                nc.tensor.matmul(out=ops[:], lhsT=s2r[:, ds], rhs=bpw[:],
                                 start=True, stop=False)
                nc.tensor.matmul(out=ops[:], lhsT=drir[:, ds], rhs=bgcn[:],
                                 start=False, stop=False)
                nc.tensor.matmul(out=ops[:], lhsT=xaggT[:, ds], rhs=wpw[:],
                                 start=False, stop=True)
                orow = opool.tile([128, HID], f32, tag="orow")
                nc.scalar.activation(out=orow[:], in_=ops[:],
                                     func=mybir.ActivationFunctionType.Copy,
                                     scale=dcol[:, g:g + 1])
                nc.sync.dma_start(
                    out=out_t.rearrange("(g p) h -> g p h", p=128)[g],
                    in_=orow[:],
                )

    nc.compile()
    return nc


def _preprocess(edge_index):
    """Host-side index preprocessing. Returns per-core index structures."""
    src = np.asarray(edge_index[0], dtype=np.int64)
    dst = np.asarray(edge_index[1], dtype=np.int64)
    loops = np.arange(N_NODES, dtype=np.int64)
    src_all = np.concatenate([src, loops])
    dst_all = np.concatenate([dst, loops])
    deg = np.bincount(dst_all, minlength=N_NODES).astype(np.float64)
    dinv = (1.0 / np.sqrt(deg)).astype(np.float32)

    # serpentine assignment of degree-sorted nodes to cores
    order = np.argsort(-deg, kind="stable")
    r = np.arange(N_NODES)
    rr = r % (2 * M)
    core_r = np.where(rr < M, rr, 2 * M - 1 - rr)
    lrank_r = (r // (2 * M)) * 2 + (rr >= M)
    core_of = np.empty(N_NODES, np.int64)
    lrank_of = np.empty(N_NODES, np.int64)
    core_of[order] = core_r
    lrank_of[order] = lrank_r

    # per-core permutation: perm[c][l] = global node at local rank l
    perm = np.empty((M, N_NODES // M), np.int64)
    perm[core_of[order], lrank_of[order]] = order

    # edges keyed by (core, block, dst_local)
    e_core = core_of[dst_all]
    e_lrank = lrank_of[dst_all]
    e_block = e_lrank // 128
    e_dl = e_lrank % 128
    key = (e_core * NGRP + e_block) * 128 + e_dl
    eorder = np.argsort(key, kind="stable")
    cnt = np.bincount(e_core * NGRP + e_block, minlength=M * NGRP).reshape(M, NGRP)
    ni_list = np.maximum(_round_up(cnt.max(axis=0), 128), 128).astype(np.int64)
    TOT = int(ni_list.sum())

    # SBUF table tokens are node ids (partition n%128, rank n//128)
    tok_of = np.arange(NP).astype(np.int16)
    dummy_tok = tok_of[N_NODES]  # a zero row

    tok_arr = np.full((M, TOT), dummy_tok, np.int16)
    dl_arr = np.full((M, TOT), DUMMY_DL, np.float16)
    goff = np.concatenate([[0], np.cumsum(ni_list)])
    s_tok = tok_of[src_all[eorder]]
    s_dl = e_dl[eorder].astype(np.float16)
    s_core = e_core[eorder]
    s_block = e_block[eorder]
    bounds = np.searchsorted(s_core * NGRP + s_block, np.arange(M * NGRP + 1), side="left")
    for c in range(M):
        for g in range(NGRP):
            b0, b1 = bounds[c * NGRP + g], bounds[c * NGRP + g + 1]
            n = b1 - b0
            # sort the group's edges by token so the gather's HBM reads
            # sweep nearly linearly (dst identity is carried by dl)
            o = np.argsort(s_tok[b0:b1], kind="stable")
            tok_arr[c, goff[g]:goff[g] + n] = s_tok[b0:b1][o]
            dl_arr[c, goff[g]:goff[g] + n] = s_dl[b0:b1][o]

    # s2[d] = sum over in-edges of dinv[src] (self loop included)
    s2 = np.bincount(dst_all, weights=dinv[src_all].astype(np.float64),
                     minlength=N_NODES).astype(np.float32)

    # wrap indices: per group, idx i -> [i%16, goff/16 + i//16]; replicate x8
    idx_w = np.zeros((M, 16, TOT // 16), np.int16)
    dl_t = np.zeros((M, 128, TOT // 128), np.float16)
    for g in range(NGRP):
        ni = int(ni_list[g])
        i = np.arange(ni)
        seg = tok_arr[:, goff[g]:goff[g] + ni]
        idx_w[:, i % 16, goff[g] // 16 + i // 16] = seg
        dseg = dl_arr[:, goff[g]:goff[g] + ni]
        dl_t[:, i % 128, goff[g] // 128 + i // 128] = dseg
    idx_rep = np.tile(idx_w, (1, 8, 1))

    return dict(dinv=dinv, perm=perm, ni_list=ni_list, idx_rep=idx_rep,
                dl_t=dl_t, s2=s2)


LAST_RESULT = None


def kernel(x, edge_index, Wp, bp, W_ih, W_hh, b_ih, b_hh, initial_weight, b_gcn):
    global LAST_RESULT
    from concourse.bass_utils import run_bass_kernel_spmd

    x = np.asarray(x, np.float32)
    Wp = np.asarray(Wp, np.float32)
    bp = np.asarray(bp, np.float32)
    W_ih = np.asarray(W_ih, np.float32)
    W_hh = np.asarray(W_hh, np.float32)
    b_ih = np.asarray(b_ih, np.float32)
    b_hh = np.asarray(b_hh, np.float32)
    initial_weight = np.asarray(initial_weight, np.float32)
    b_gcn = np.asarray(b_gcn, np.float32)
    assert x.shape == (N_NODES, IN_DIM)

    pre = _preprocess(edge_index)
    dinv, perm, ni_list, s2 = pre["dinv"], pre["perm"], pre["ni_list"], pre["s2"]

    key = tuple(int(v) for v in ni_list)
    if key not in _cache:
        _cache[key] = _build_module(key)
    nc = _cache[key]

    # shared tensors: dinv-scaled fp16 source table, token layout
    # (partition n%128, rank n//128)
    xsp = np.zeros((NP, IN_DIM), np.float32)
    xsp[:N_NODES] = x * dinv[:, None]
    xs_tiled = np.ascontiguousarray(
        xsp.reshape(RANKS, 128, IN_DIM).transpose(1, 0, 2)
        .reshape(128, RANKS * 128)).astype(np.float16)
    iota_np = np.tile(np.arange(128, dtype=np.float16)[None, :], (128, 1))
    shared = {
        "xs_tiled": xs_tiled,
        "iota": np.ascontiguousarray(iota_np),
        "WsumT": np.ascontiguousarray((W_ih + W_hh).T),
        "IW": initial_weight,
        "IWT": np.ascontiguousarray(initial_weight.T),
        "WpT": np.ascontiguousarray(Wp.T),
        "bsum": (b_ih + b_hh).reshape(1, -1),
        "bp_col": np.ascontiguousarray(bp.reshape(-1, 1)),
        "b_gcn": b_gcn.reshape(1, -1),
        "ones_row": np.ones((1, 128), np.float32),
    }
    NLOC = N_NODES // M
    in_maps = []
    for c in range(M):
        pc = perm[c]
        s2p = np.zeros(NPC, np.float32)
        s2p[:NLOC] = s2[pc]
        drip = np.zeros(NPC, np.float32)
        drip[:NLOC] = 1.0 / dinv[pc]
        dlocp = np.zeros(NPC, np.float32)
        dlocp[:NLOC] = dinv[pc]
        in_maps.append({
            **shared,
            "idx": np.ascontiguousarray(pre["idx_rep"][c]),
            "dl": np.ascontiguousarray(pre["dl_t"][c]),
            "s2_row": s2p.reshape(1, -1),
            "dri_row": drip.reshape(1, -1),
            "dinv_col": np.ascontiguousarray(dlocp.reshape(NGRP, 128).T),
        })

    res = run_bass_kernel_spmd(nc, in_maps, list(range(M)))
    LAST_RESULT = res

    out = np.empty((N_NODES, HID), np.float32)
    for c in range(M):
        out[perm[c]] = res.results[c]["out"][:NLOC]
    return out


# revision 7
# speedup vs baseline: 1.9321x; 1.9321x over previous
"""Trainium2 Bass kernel for EvolveGCN-O forward (GCN message passing).

Math (reference):
    h   = x @ Wp + bp
    W   = LSTM-evolved weight from initial_weight (one step, h0=c0=IW)
    hw  = h @ W
    out = D^-1/2 (A+I) D^-1/2 hw + b_gcn

Factored for the kernel:
    out[d] = dinv[d] * (sum_{e: dst=d} dinv[src_e] * x[src_e]) @ (Wp @ W)
             + s2[d]*dinv[d]*(bp @ W) + b_gcn
with s2[d] = sum_{e in(d)} dinv[src_e] (self loops included as edges).

Distribution: nodes (dsts) sharded over 8 NeuronCores (serpentine by degree);
the dinv-scaled fp16 source-feature table is replicated in every core's SBUF;
each core gathers its edges' source rows via SWDGE prepare-only descriptor
generation + trigger_dma on 4 queues (the prep never blocks the GpSimd engine
on transfer completion), performs the segment sum on the TensorEngine via
one-hot masks (built on GpSimd/Vector), then applies the (tiny, replicated)
evolved-weight matmul.
"""

import numpy as np

N_NODES = 10000
N_EDGES = 320000
IN_DIM = 128
HID = 256
M = 8                    # NeuronCores
NP = 10240               # padded node count (mult of 128)
RANKS = NP // 128        # 80
NPC = NP // M            # 1280 padded dsts per core
NGRP = NPC // 128        # 10 dst blocks of 128 per core
DUMMY_DL = 999.0         # dst_local value that matches no column

_cache = {}


def _round_up(x, m):
    return (x + m - 1) // m * m


def _build_module(ni_list):
    """Build+compile the Bacc module for given per-group edge counts."""
    import concourse.bacc as bacc
    import concourse.mybir as mybir
    import concourse.tile as tile

    TOT = int(sum(ni_list))
    nc = bacc.Bacc("TRN2", target_bir_lowering=False, debug=False,
                   num_devices=M, num_swdge_queues=4)
    f32, f16, i16 = mybir.dt.float32, mybir.dt.float16, mybir.dt.int16

    # ---- DRAM inputs ----
    xs_in = nc.dram_tensor("xs_tiled", [128, RANKS * 128], f16, kind="ExternalInput").ap()
    idx_in = nc.dram_tensor("idx", [128, TOT // 16], i16, kind="ExternalInput").ap()
    dl_in = nc.dram_tensor("dl", [128, TOT // 128], f16, kind="ExternalInput").ap()
    iota_in = nc.dram_tensor("iota", [128, 128], f16, kind="ExternalInput").ap()
    ws_in = nc.dram_tensor("WsumT", [256, 1024], f32, kind="ExternalInput").ap()
    iw_in = nc.dram_tensor("IW", [256, 256], f32, kind="ExternalInput").ap()
    iwt_in = nc.dram_tensor("IWT", [256, 256], f32, kind="ExternalInput").ap()
    wpt_in = nc.dram_tensor("WpT", [256, 128], f32, kind="ExternalInput").ap()
    bsum_in = nc.dram_tensor("bsum", [1, 1024], f32, kind="ExternalInput").ap()
    bp_in = nc.dram_tensor("bp_col", [256, 1], f32, kind="ExternalInput").ap()
    bgcn_in = nc.dram_tensor("b_gcn", [1, 256], f32, kind="ExternalInput").ap()
    ones_in = nc.dram_tensor("ones_row", [1, 128], f32, kind="ExternalInput").ap()
    s2_in = nc.dram_tensor("s2_row", [1, NPC], f32, kind="ExternalInput").ap()
    dri_in = nc.dram_tensor("dri_row", [1, NPC], f32, kind="ExternalInput").ap()
    dcol_in = nc.dram_tensor("dinv_col", [128, NGRP], f32, kind="ExternalInput").ap()

    out_t = nc.dram_tensor("out", [NPC, HID], f32, kind="ExternalOutput").ap()

    def gather_sbuf_nt(out_ap, in_ap, idxs_ap, num_idxs, queue_num):
        """SBUF-source non-transpose SWDGE gather. The ucode blocks in the
        instruction until the queue's previous DMA drains, so pipelining
        depth equals the number of queues."""
        eng = nc.gpsimd
        in_ap = in_ap.bitcast(out_ap.dtype) if in_ap.dtype != out_ap.dtype else in_ap
        return eng.add_instruction(
            mybir.InstDMAGatherAnt(
                name=f"I-{nc.next_id()}",
                ins=[eng.lower_ap(in_ap), eng.lower_ap(idxs_ap),
                     eng.lower_val_access(eng.to_reg(num_idxs))],
                outs=[eng.lower_ap(out_ap)],
                transpose=False,
                num_idxs=num_idxs,
                elem_size=128,
                stride_bytes_256=0,
                gen_mode=0,
                single_packet=False,
                queue_num=queue_num,
                sbuf_tokens_per_rank=128,
                sbuf_free_dim_per_rank=256,
                sbuf_free_dim_pad_per_rank=0,
                sbuf_byte_offset=0,
            )
        )

    with tile.TileContext(nc) as tc:
        with (
            tc.tile_pool(name="stage", bufs=1) as stpool,
            tc.tile_pool(name="persist", bufs=1) as pp,
            tc.tile_pool(name="sp", bufs=3) as spool,
            tc.tile_pool(name="op", bufs=3) as opool,
            tc.tile_pool(name="psg", bufs=2, space="PSUM") as psg,
            tc.tile_pool(name="psl", bufs=1, space="PSUM") as psl,
        ):
            # ---------- phase 0: table + index loads ----------
            xs_sb = pp.tile([128, RANKS * 128], f16)
            idxs = pp.tile([128, TOT // 16], i16)
            dls = pp.tile([128, TOT // 128], f16)
            iota = pp.tile([128, 128], f16)
            HLF = RANKS * 64
            nc.sync.dma_start(out=xs_sb[:, :HLF], in_=xs_in[:, :HLF])
            nc.scalar.dma_start(out=idxs[:], in_=idx_in[:])
            nc.scalar.dma_start(out=xs_sb[:, HLF:], in_=xs_in[:, HLF:])
            nc.sync.dma_start(out=dls[:], in_=dl_in[:])
            nc.sync.dma_start(out=iota[:], in_=iota_in[:])

            # ---------- gathers: prep + trigger everything up front ----------
            arena = pp.tile([128, TOT // 128, 128], f16)
            goff = [0]
            for ni in ni_list:
                goff.append(goff[-1] + int(ni))
            for g in range(NGRP):
                ni = int(ni_list[g])
                base = goff[g]
                gather_sbuf_nt(
                    arena[:, base // 128:(base + ni) // 128, :],
                    xs_sb[:],
                    idxs[:, base // 16:(base + ni) // 16],
                    ni,
                    g % 4,
                )

            # ---------- small-tensor loads + LSTM weight evolution ----------
            wsum = pp.tile([128, 2, 1024], f32)
            iw = pp.tile([128, 2, 256], f32)
            iwt = pp.tile([128, 2, 256], f32)
            wpt = pp.tile([128, 2, 128], f32)
            bsum = pp.tile([1, 1024], f32)
            bp_c = pp.tile([128, 2, 1], f32)
            bgcn = pp.tile([1, 256], f32)
            ones = pp.tile([1, 128], f32)
            s2r = pp.tile([1, NPC], f32)
            drir = pp.tile([1, NPC], f32)
            dcol = pp.tile([128, NGRP], f32)
            for t_, s_ in ((wsum, ws_in), (iw, iw_in), (iwt, iwt_in),
                           (wpt, wpt_in), (bp_c, bp_in)):
                nc.scalar.dma_start(out=t_[:], in_=s_.rearrange("(k p) c -> p k c", p=128))
            for t_, s_ in ((bsum, bsum_in), (bgcn, bgcn_in), (ones, ones_in),
                           (s2r, s2_in), (dri_in_t := drir, dri_in),
                           (dcol, dcol_in)):
                nc.sync.dma_start(out=t_[:], in_=s_[:])

            w_ev = pp.tile([128, 2, 256], f32)   # evolved GCN weight W
            Sig = mybir.ActivationFunctionType.Sigmoid
            Tanh = mybir.ActivationFunctionType.Tanh
            for ic in range(2):
                gpsum = psl.tile([128, 1024], f32, space="PSUM", tag="gates")
                for h in range(2):
                    gs = slice(512 * h, 512 * (h + 1))
                    nc.tensor.matmul(out=gpsum[:, gs], lhsT=ones[:, :],
                                     rhs=bsum[:, gs], start=True, stop=False)
                    nc.tensor.matmul(out=gpsum[:, gs],
                                     lhsT=iwt[:, 0, 128 * ic:128 * (ic + 1)],
                                     rhs=wsum[:, 0, gs], start=False, stop=False)
                    nc.tensor.matmul(out=gpsum[:, gs],
                                     lhsT=iwt[:, 1, 128 * ic:128 * (ic + 1)],
                                     rhs=wsum[:, 1, gs], start=False, stop=True)
                si = stpool.tile([128, 256], f32, tag="si")
                sf = stpool.tile([128, 256], f32, tag="sf")
                tg = stpool.tile([128, 256], f32, tag="tg")
                so = stpool.tile([128, 256], f32, tag="so")
                nc.scalar.activation(out=si[:], in_=gpsum[:, 0:256], func=Sig)
                nc.scalar.activation(out=sf[:], in_=gpsum[:, 256:512], func=Sig)
                nc.scalar.activation(out=tg[:], in_=gpsum[:, 512:768], func=Tanh)
                nc.scalar.activation(out=so[:], in_=gpsum[:, 768:1024], func=Sig)
                c1 = stpool.tile([128, 256], f32, tag="c1")
                nc.vector.tensor_tensor(out=c1[:], in0=sf[:], in1=iw[:, ic, :],
                                        op=mybir.AluOpType.mult)
                c2 = stpool.tile([128, 256], f32, tag="c2")
                nc.vector.tensor_tensor(out=c2[:], in0=si[:], in1=tg[:],
                                        op=mybir.AluOpType.mult)
                cc = stpool.tile([128, 256], f32, tag="cc")
                nc.vector.tensor_tensor(out=cc[:], in0=c1[:], in1=c2[:],
                                        op=mybir.AluOpType.add)
                tcc = stpool.tile([128, 256], f32, tag="tcc")
                nc.scalar.activation(out=tcc[:], in_=cc[:], func=Tanh)
                nc.vector.tensor_tensor(out=w_ev[:, ic, :], in0=so[:], in1=tcc[:],
                                        op=mybir.AluOpType.mult)

            wpw = pp.tile([128, 256], f32)       # Wp @ W
            wp_ps = psl.tile([128, 256], f32, space="PSUM", tag="wpw")
            nc.tensor.matmul(out=wp_ps[:], lhsT=wpt[:, 0, :], rhs=w_ev[:, 0, :],
                             start=True, stop=False)
            nc.tensor.matmul(out=wp_ps[:], lhsT=wpt[:, 1, :], rhs=w_ev[:, 1, :],
                             start=False, stop=True)
            nc.vector.tensor_copy(out=wpw[:], in_=wp_ps[:])
            bpw = pp.tile([1, 256], f32)         # bp @ W
            bp_ps = psl.tile([1, 256], f32, space="PSUM", tag="bpw")
            nc.tensor.matmul(out=bp_ps[:], lhsT=bp_c[:, 0, :], rhs=w_ev[:, 0, :],
                             start=True, stop=False)
            nc.tensor.matmul(out=bp_ps[:], lhsT=bp_c[:, 1, :], rhs=w_ev[:, 1, :],
                             start=False, stop=True)
            nc.vector.tensor_copy(out=bpw[:], in_=bp_ps[:])

            # ---------- main: one-hot masks + PE segment sum + output ----------
            xaggT = pp.tile([128, NPC], f32)
            for g in range(NGRP):
                ni = int(ni_list[g])
                nch = ni // 128
                base = goff[g]
                coff = base // 128
                # one-hot mask: st[p, c, d] = (dl[p, c] == d)
                st = spool.tile([128, nch, 128], f16, tag="st")
                nc.vector.tensor_tensor(
                    out=st[:],
                    in0=dls[:, coff:coff + nch]
                        .rearrange("p (c o) -> p c o", o=1)
                        .to_broadcast([128, nch, 128]),
                    in1=iota[:].rearrange("p (o d) -> p o d", o=1)
                        .to_broadcast([128, nch, 128]),
                    op=mybir.AluOpType.is_equal,
                )
                gps = psg.tile([128, 128], f32, space="PSUM", tag="agg")
                for c in range(nch):
                    nc.tensor.matmul(out=gps[:], lhsT=arena[:, coff + c, :],
                                     rhs=st[:, c, :],
                                     start=(c == 0), stop=(c == nch - 1))
                nc.scalar.activation(out=xaggT[:, g * 128:(g + 1) * 128],
                                     in_=gps[:],
                                     func=mybir.ActivationFunctionType.Copy)

                # final: out rows = dinv*(xagg@WpW + s2*bpW + dri*bgcn)
                ops = psg.tile([128, HID], f32, space="PSUM", tag="ops")
                ds = slice(128 * g, 128 * (g + 1))
                nc.tensor.matmul(out=ops[:], lhsT=s2r[:, ds], rhs=bpw[:],
                                 start=True, stop=False)
                nc.tensor.matmul(out=ops[:], lhsT=drir[:, ds], rhs=bgcn[:],
                                 start=False, stop=False)
                nc.tensor.matmul(out=ops[:], lhsT=xaggT[:, ds], rhs=wpw[:],
                                 start=False, stop=True)
                orow = opool.tile([128, HID], f32, tag="orow")
                nc.scalar.activation(out=orow[:], in_=ops[:],
                                     func=mybir.ActivationFunctionType.Copy,
                                     scale=dcol[:, g:g + 1])
                nc.sync.dma_start(
                    out=out_t.rearrange("(g p) h -> g p h", p=128)[g],
                    in_=orow[:],
                )

    nc.compile()
    return nc


def _preprocess(edge_index):
    """Host-side index preprocessing. Returns per-core index structures."""
    src = np.asarray(edge_index[0], dtype=np.int64)
    dst = np.asarray(edge_index[1], dtype=np.int64)
    loops = np.arange(N_NODES, dtype=np.int64)
    src_all = np.concatenate([src, loops])
    dst_all = np.concatenate([dst, loops])
    deg = np.bincount(dst_all, minlength=N_NODES).astype(np.float64)
    dinv = (1.0 / np.sqrt(deg)).astype(np.float32)

    # serpentine assignment of degree-sorted nodes to cores
    order = np.argsort(-deg, kind="stable")
    r = np.arange(N_NODES)
    rr = r % (2 * M)
    core_r = np.where(rr < M, rr, 2 * M - 1 - rr)
    lrank_r = (r // (2 * M)) * 2 + (rr >= M)
    core_of = np.empty(N_NODES, np.int64)
    lrank_of = np.empty(N_NODES, np.int64)
    core_of[order] = core_r
    lrank_of[order] = lrank_r

    # per-core permutation: perm[c][l] = global node at local rank l
    perm = np.empty((M, N_NODES // M), np.int64)
    perm[core_of[order], lrank_of[order]] = order

    # edges keyed by (core, block, dst_local)
    e_core = core_of[dst_all]
    e_lrank = lrank_of[dst_all]
    e_block = e_lrank // 128
    e_dl = e_lrank % 128
    key = (e_core * NGRP + e_block) * 128 + e_dl
    eorder = np.argsort(key, kind="stable")
    cnt = np.bincount(e_core * NGRP + e_block, minlength=M * NGRP).reshape(M, NGRP)
    ni_list = np.maximum(_round_up(cnt.max(axis=0), 128), 128).astype(np.int64)
    TOT = int(ni_list.sum())

    # SBUF table tokens are node ids (partition n%128, rank n//128)
    tok_of = np.arange(NP).astype(np.int16)
    dummy_tok = tok_of[N_NODES]  # a zero row

    tok_arr = np.full((M, TOT), dummy_tok, np.int16)
    dl_arr = np.full((M, TOT), DUMMY_DL, np.float16)
    goff = np.concatenate([[0], np.cumsum(ni_list)])
    s_tok = tok_of[src_all[eorder]]
    s_dl = e_dl[eorder].astype(np.float16)
    s_core = e_core[eorder]
    s_block = e_block[eorder]
    bounds = np.searchsorted(s_core * NGRP + s_block, np.arange(M * NGRP + 1), side="left")
    for c in range(M):
        for g in range(NGRP):
            b0, b1 = bounds[c * NGRP + g], bounds[c * NGRP + g + 1]
            n = b1 - b0
            # sort the group's edges by token so the gather's HBM reads
            # sweep nearly linearly (dst identity is carried by dl)
            o = np.argsort(s_tok[b0:b1], kind="stable")
            tok_arr[c, goff[g]:goff[g] + n] = s_tok[b0:b1][o]
            dl_arr[c, goff[g]:goff[g] + n] = s_dl[b0:b1][o]

    # s2[d] = sum over in-edges of dinv[src] (self loop included)
    s2 = np.bincount(dst_all, weights=dinv[src_all].astype(np.float64),
                     minlength=N_NODES).astype(np.float32)

    # wrap indices: per group, idx i -> [i%16, goff/16 + i//16]; replicate x8
    idx_w = np.zeros((M, 16, TOT // 16), np.int16)
    dl_t = np.zeros((M, 128, TOT // 128), np.float16)
    for g in range(NGRP):
        ni = int(ni_list[g])
        i = np.arange(ni)
        seg = tok_arr[:, goff[g]:goff[g] + ni]
        idx_w[:, i % 16, goff[g] // 16 + i // 16] = seg
        dseg = dl_arr[:, goff[g]:goff[g] + ni]
        dl_t[:, i % 128, goff[g] // 128 + i // 128] = dseg
    idx_rep = np.tile(idx_w, (1, 8, 1))

    return dict(dinv=dinv, perm=perm, ni_list=ni_list, idx_rep=idx_rep,
                dl_t=dl_t, s2=s2)


LAST_RESULT = None


def kernel(x, edge_index, Wp, bp, W_ih, W_hh, b_ih, b_hh, initial_weight, b_gcn):
    global LAST_RESULT
    from concourse.bass_utils import run_bass_kernel_spmd

    x = np.asarray(x, np.float32)
    Wp = np.asarray(Wp, np.float32)
    bp = np.asarray(bp, np.float32)
    W_ih = np.asarray(W_ih, np.float32)
    W_hh = np.asarray(W_hh, np.float32)
    b_ih = np.asarray(b_ih, np.float32)
    b_hh = np.asarray(b_hh, np.float32)
    initial_weight = np.asarray(initial_weight, np.float32)
    b_gcn = np.asarray(b_gcn, np.float32)
    assert x.shape == (N_NODES, IN_DIM)

    pre = _preprocess(edge_index)
    dinv, perm, ni_list, s2 = pre["dinv"], pre["perm"], pre["ni_list"], pre["s2"]

    key = tuple(int(v) for v in ni_list)
    if key not in _cache:
        _cache[key] = _build_module(key)
    nc = _cache[key]

    # shared tensors: dinv-scaled fp16 source table, token layout
    # (partition n%128, rank n//128)
    xsp = np.zeros((NP, IN_DIM), np.float32)
    xsp[:N_NODES] = x * dinv[:, None]
    xs_tiled = np.ascontiguousarray(
        xsp.reshape(RANKS, 128, IN_DIM).transpose(1, 0, 2)
        .reshape(128, RANKS * 128)).astype(np.float16)
    iota_np = np.tile(np.arange(128, dtype=np.float16)[None, :], (128, 1))
    shared = {
        "xs_tiled": xs_tiled,
        "iota": np.ascontiguousarray(iota_np),
        "WsumT": np.ascontiguousarray((W_ih + W_hh).T),
        "IW": initial_weight,
        "IWT": np.ascontiguousarray(initial_weight.T),
        "WpT": np.ascontiguousarray(Wp.T),
        "bsum": (b_ih + b_hh).reshape(1, -1),
        "bp_col": np.ascontiguousarray(bp.reshape(-1, 1)),
        "b_gcn": b_gcn.reshape(1, -1),
        "ones_row": np.ones((1, 128), np.float32),
    }
    NLOC = N_NODES // M
    in_maps = []
    for c in range(M):
        pc = perm[c]
        s2p = np.zeros(NPC, np.float32)
        s2p[:NLOC] = s2[pc]
        drip = np.zeros(NPC, np.float32)
        drip[:NLOC] = 1.0 / dinv[pc]
        dlocp = np.zeros(NPC, np.float32)
        dlocp[:NLOC] = dinv[pc]
        in_maps.append({
            **shared,
            "idx": np.ascontiguousarray(pre["idx_rep"][c]),
            "dl": np.ascontiguousarray(pre["dl_t"][c]),
            "s2_row": s2p.reshape(1, -1),
            "dri_row": drip.reshape(1, -1),
            "dinv_col": np.ascontiguousarray(dlocp.reshape(NGRP, 128).T),
        })

    res = run_bass_kernel_spmd(nc, in_maps, list(range(M)))
    LAST_RESULT = res

    out = np.empty((N_NODES, HID), np.float32)
    for c in range(M):
        out[perm[c]] = res.results[c]["out"][:NLOC]
    return out


# revision 9
# speedup vs baseline: 2.6902x; 1.3923x over previous
"""Trainium2 Bass kernel for EvolveGCN-O forward (GCN message passing).

Math (reference):
    h   = x @ Wp + bp
    W   = LSTM-evolved weight from initial_weight (one step, h0=c0=IW)
    hw  = h @ W
    out = D^-1/2 (A+I) D^-1/2 hw + b_gcn

Factored for the kernel:
    out[d] = dinv[d] * (sum_{e: dst=d} dinv[src_e] * x[src_e]) @ (Wp @ W)
             + s2[d]*dinv[d]*(bp @ W) + b_gcn
with s2[d] = sum_{e in(d)} dinv[src_e] (self loops included as edges).

Distribution: nodes (dsts) sharded over 8 NeuronCores (serpentine by degree).
The aggregation over in-edges is computed as a dense blocked matmul: for each
source rank r (128 nodes), xaggT[:, :] += XsT_r @ M_r where Xs is the
dinv-scaled fp16 source-feature table (replicated) and M_r is the fp8 block of
the edge-multiplicity matrix (src-rank r x this core's 1280 dsts; counts are
small integers, exact in fp8). M is streamed from HBM in rank chunks while the
TensorEngine accumulates all 80 ranks into PSUM; no per-edge DMA is needed.
The tiny [H,H] LSTM weight evolution is replicated on every core.
"""

import numpy as np

N_NODES = 10000
N_EDGES = 320000
IN_DIM = 128
HID = 256
M = 8                    # NeuronCores
NP = 10240               # padded node count (mult of 128)
RANKS = NP // 128        # 80
NPC = NP // M            # 1280 padded dsts per core
NGRP = NPC // 128        # 10 dst blocks of 128 per core
RCH = 8                  # ranks per streamed M chunk
NCHK = RANKS // RCH      # 10 chunks

_cache = {}


def _build_module():
    """Build+compile the Bacc module (shapes are static)."""
    import concourse.bacc as bacc
    import concourse.mybir as mybir
    import concourse.tile as tile

    nc = bacc.Bacc("TRN2", target_bir_lowering=False, debug=False,
                   num_devices=M)
    f32, f16, f8 = mybir.dt.float32, mybir.dt.float16, mybir.dt.float8e4

    # ---- DRAM inputs ----
    xs_in = nc.dram_tensor("xs_tiled", [128, RANKS * 128], f16, kind="ExternalInput").ap()
    m_in = nc.dram_tensor("Mt", [128, RANKS * NPC], f8, kind="ExternalInput").ap()
    ws_in = nc.dram_tensor("WsumT", [256, 1024], f32, kind="ExternalInput").ap()
    iw_in = nc.dram_tensor("IW", [256, 256], f32, kind="ExternalInput").ap()
    iwt_in = nc.dram_tensor("IWT", [256, 256], f32, kind="ExternalInput").ap()
    wpt_in = nc.dram_tensor("WpT", [256, 128], f32, kind="ExternalInput").ap()
    bsum_in = nc.dram_tensor("bsum", [1, 1024], f32, kind="ExternalInput").ap()
    bp_in = nc.dram_tensor("bp_col", [256, 1], f32, kind="ExternalInput").ap()
    bgcn_in = nc.dram_tensor("b_gcn", [1, 256], f32, kind="ExternalInput").ap()
    ones_in = nc.dram_tensor("ones_row", [1, 128], f32, kind="ExternalInput").ap()
    s2_in = nc.dram_tensor("s2_row", [1, NPC], f32, kind="ExternalInput").ap()
    dri_in = nc.dram_tensor("dri_row", [1, NPC], f32, kind="ExternalInput").ap()
    dcol_in = nc.dram_tensor("dinv_col", [128, NGRP], f32, kind="ExternalInput").ap()

    out_t = nc.dram_tensor("out", [NPC, HID], f32, kind="ExternalOutput").ap()

    with tile.TileContext(nc) as tc:
        with (
            tc.tile_pool(name="stage", bufs=1) as stpool,
            tc.tile_pool(name="persist", bufs=1) as pp,
            tc.tile_pool(name="mp", bufs=3) as mpool,
            tc.tile_pool(name="op", bufs=2) as opool,
            tc.tile_pool(name="psa", bufs=1, space="PSUM") as psa,
            tc.tile_pool(name="psg", bufs=1, space="PSUM") as psg,
            tc.tile_pool(name="psl", bufs=1, space="PSUM") as psl,
        ):
            # ---------- loads: weights first (gates need them), then table+M ----
            bsum = pp.tile([1, 1024], f32)
            bgcn = pp.tile([1, 256], f32)
            ones = pp.tile([1, 128], f32)
            nc.sync.dma_start(out=bsum[:], in_=bsum_in[:])
            nc.sync.dma_start(out=bgcn[:], in_=bgcn_in[:])
            nc.sync.dma_start(out=ones[:], in_=ones_in[:])
            wsum = pp.tile([128, 2, 1024], f32)
            iwt = pp.tile([128, 2, 256], f32)
            iw = pp.tile([128, 2, 256], f32)
            wpt = pp.tile([128, 2, 128], f32)
            bp_c = pp.tile([128, 2, 1], f32)
            nc.scalar.dma_start(out=wsum[:], in_=ws_in.rearrange("(k p) c -> p k c", p=128))
            nc.scalar.dma_start(out=iwt[:], in_=iwt_in.rearrange("(k p) c -> p k c", p=128))

            xs_sb = pp.tile([128, RANKS * 128], f16)
            nc.sync.dma_start(out=xs_sb[:], in_=xs_in[:])

            nc.scalar.dma_start(out=iw[:], in_=iw_in.rearrange("(k p) c -> p k c", p=128))
            nc.scalar.dma_start(out=wpt[:], in_=wpt_in.rearrange("(k p) c -> p k c", p=128))
            nc.scalar.dma_start(out=bp_c[:], in_=bp_in.rearrange("(k p) c -> p k c", p=128))
            s2r = pp.tile([1, NPC], f32)
            drir = pp.tile([1, NPC], f32)
            dcol = pp.tile([128, NGRP], f32)
            nc.sync.dma_start(out=s2r[:], in_=s2_in[:])
            nc.sync.dma_start(out=drir[:], in_=dri_in[:])
            nc.sync.dma_start(out=dcol[:], in_=dcol_in[:])

            # ---------- LSTM weight evolution (tiny, replicated) ----------
            w_ev = pp.tile([128, 2, 256], f32)   # evolved GCN weight W
            Sig = mybir.ActivationFunctionType.Sigmoid
            Tanh = mybir.ActivationFunctionType.Tanh
            for ic in range(2):
                gpsum = psl.tile([128, 1024], f32, space="PSUM", tag="gates")
                for h in range(2):
                    gs = slice(512 * h, 512 * (h + 1))
                    nc.tensor.matmul(out=gpsum[:, gs], lhsT=ones[:, :],
                                     rhs=bsum[:, gs], start=True, stop=False)
                    nc.tensor.matmul(out=gpsum[:, gs],
                                     lhsT=iwt[:, 0, 128 * ic:128 * (ic + 1)],
                                     rhs=wsum[:, 0, gs], start=False, stop=False)
                    nc.tensor.matmul(out=gpsum[:, gs],
                                     lhsT=iwt[:, 1, 128 * ic:128 * (ic + 1)],
                                     rhs=wsum[:, 1, gs], start=False, stop=True)
                si = stpool.tile([128, 256], f32, tag="si")
                sf = stpool.tile([128, 256], f32, tag="sf")
                tg = stpool.tile([128, 256], f32, tag="tg")
                so = stpool.tile([128, 256], f32, tag="so")
                nc.scalar.activation(out=si[:], in_=gpsum[:, 0:256], func=Sig)
                nc.scalar.activation(out=sf[:], in_=gpsum[:, 256:512], func=Sig)
                nc.scalar.activation(out=tg[:], in_=gpsum[:, 512:768], func=Tanh)
                nc.scalar.activation(out=so[:], in_=gpsum[:, 768:1024], func=Sig)
                c1 = stpool.tile([128, 256], f32, tag="c1")
                nc.vector.tensor_tensor(out=c1[:], in0=sf[:], in1=iw[:, ic, :],
                                        op=mybir.AluOpType.mult)
                c2 = stpool.tile([128, 256], f32, tag="c2")
                nc.vector.tensor_tensor(out=c2[:], in0=si[:], in1=tg[:],
                                        op=mybir.AluOpType.mult)
                cc = stpool.tile([128, 256], f32, tag="cc")
                nc.vector.tensor_tensor(out=cc[:], in0=c1[:], in1=c2[:],
                                        op=mybir.AluOpType.add)
                tcc = stpool.tile([128, 256], f32, tag="tcc")
                nc.scalar.activation(out=tcc[:], in_=cc[:], func=Tanh)
                nc.vector.tensor_tensor(out=w_ev[:, ic, :], in0=so[:], in1=tcc[:],
                                        op=mybir.AluOpType.mult)

            wpw = pp.tile([128, 256], f32)       # Wp @ W
            wp_ps = psl.tile([128, 256], f32, space="PSUM", tag="wpw")
            nc.tensor.matmul(out=wp_ps[:], lhsT=wpt[:, 0, :], rhs=w_ev[:, 0, :],
                             start=True, stop=False)
            nc.tensor.matmul(out=wp_ps[:], lhsT=wpt[:, 1, :], rhs=w_ev[:, 1, :],
                             start=False, stop=True)
            nc.vector.tensor_copy(out=wpw[:], in_=wp_ps[:])
            bpw = pp.tile([1, 256], f32)         # bp @ W
            bp_ps = psl.tile([1, 256], f32, space="PSUM", tag="bpw")
            nc.tensor.matmul(out=bp_ps[:], lhsT=bp_c[:, 0, :], rhs=w_ev[:, 0, :],
                             start=True, stop=False)
            nc.tensor.matmul(out=bp_ps[:], lhsT=bp_c[:, 1, :], rhs=w_ev[:, 1, :],
                             start=False, stop=True)
            nc.vector.tensor_copy(out=bpw[:], in_=bp_ps[:])

            # ---------- main: stream M, accumulate xaggT = sum_r XsT_r @ M_r ----
            pa0 = psa.tile([128, 512], f32, space="PSUM", tag="pa0")
            pa1 = psa.tile([128, 512], f32, space="PSUM", tag="pa1")
            pa2 = psa.tile([128, 256], f32, space="PSUM", tag="pa2")
            pa = [pa0, pa1, pa2]
            spans = [(0, 512), (512, 1024), (1024, 1280)]
            for c in range(NCHK):
                mt = mpool.tile([128, RCH, NPC], f8, tag="mt")
                eng = nc.sync if c % 2 == 0 else nc.scalar
                eng.dma_start(
                    out=mt[:],
                    in_=m_in[:, c * RCH * NPC:(c + 1) * RCH * NPC]
                        .rearrange("p (r d) -> p r d", d=NPC))
                for k in range(RCH):
                    r = c * RCH + k
                    lhsT = xs_sb[:, r * 128:(r + 1) * 128]
                    for t in range(3):
                        lo, hi = spans[t]
                        nc.tensor.matmul(out=pa[t][:], lhsT=lhsT,
                                         rhs=mt[:, k, lo:hi],
                                         start=(r == 0), stop=(r == RANKS - 1))

            xagg = pp.tile([128, NPC], f32)
            for t in range(3):
                lo, hi = spans[t]
                nc.scalar.activation(out=xagg[:, lo:hi], in_=pa[t][:],
                                     func=mybir.ActivationFunctionType.Copy)

            # ---------- epilogue: out rows = dinv*(xagg@WpW + s2*bpW + dri*bgcn) --
            for g in range(NGRP):
                ops = psg.tile([128, HID], f32, space="PSUM", tag="ops")
                ds = slice(128 * g, 128 * (g + 1))
                nc.tensor.matmul(out=ops[:], lhsT=s2r[:, ds], rhs=bpw[:],
                                 start=True, stop=False)
                nc.tensor.matmul(out=ops[:], lhsT=drir[:, ds], rhs=bgcn[:],
                                 start=False, stop=False)
                nc.tensor.matmul(out=ops[:], lhsT=xagg[:, ds], rhs=wpw[:],
                                 start=False, stop=True)
                orow = opool.tile([128, HID], f32, tag="orow")
                nc.scalar.activation(out=orow[:], in_=ops[:],
                                     func=mybir.ActivationFunctionType.Copy,
                                     scale=dcol[:, g:g + 1])
                nc.sync.dma_start(
                    out=out_t.rearrange("(g p) h -> g p h", p=128)[g],
                    in_=orow[:],
                )

    nc.compile()
    return nc


def _preprocess(edge_index):
    """Host-side graph preprocessing: degrees, serpentine dst sharding, and
    the per-core fp8 edge-multiplicity matrices."""
    import ml_dtypes

    src = np.asarray(edge_index[0], dtype=np.int64)
    dst = np.asarray(edge_index[1], dtype=np.int64)
    loops = np.arange(N_NODES, dtype=np.int64)
    src_all = np.concatenate([src, loops])
    dst_all = np.concatenate([dst, loops])
    deg = np.bincount(dst_all, minlength=N_NODES).astype(np.float64)
    dinv = (1.0 / np.sqrt(deg)).astype(np.float32)

    # serpentine assignment of degree-sorted nodes to cores
    order = np.argsort(-deg, kind="stable")
    r = np.arange(N_NODES)
    rr = r % (2 * M)
    core_r = np.where(rr < M, rr, 2 * M - 1 - rr)
    lrank_r = (r // (2 * M)) * 2 + (rr >= M)
    core_of = np.empty(N_NODES, np.int64)
    lrank_of = np.empty(N_NODES, np.int64)
    core_of[order] = core_r
    lrank_of[order] = lrank_r

    # per-core permutation: perm[c][l] = global node at local rank l
    perm = np.empty((M, N_NODES // M), np.int64)
    perm[core_of[order], lrank_of[order]] = order

    # per-core fp8 multiplicity matrix Mt[p, r*NPC + d] = #edges (128r+p -> d)
    e_core = core_of[dst_all]
    e_dl = lrank_of[dst_all]
    lut = np.arange(256).astype(ml_dtypes.float8_e4m3)
    Mts = []
    for c in range(M):
        sel = e_core == c
        tok = src_all[sel]
        dl = e_dl[sel]
        mt_u8 = np.zeros((128, RANKS * NPC), np.uint8)
        np.add.at(mt_u8, (tok % 128, (tok // 128) * NPC + dl), 1)
        Mts.append(lut[mt_u8])

    # s2[d] = sum over in-edges of dinv[src] (self loop included)
    s2 = np.bincount(dst_all, weights=dinv[src_all].astype(np.float64),
                     minlength=N_NODES).astype(np.float32)

    return dict(dinv=dinv, perm=perm, Mts=Mts, s2=s2)


LAST_RESULT = None


def kernel(x, edge_index, Wp, bp, W_ih, W_hh, b_ih, b_hh, initial_weight, b_gcn):
    global LAST_RESULT
    from concourse.bass_utils import run_bass_kernel_spmd

    x = np.asarray(x, np.float32)
    Wp = np.asarray(Wp, np.float32)
    bp = np.asarray(bp, np.float32)
    W_ih = np.asarray(W_ih, np.float32)
    W_hh = np.asarray(W_hh, np.float32)
    b_ih = np.asarray(b_ih, np.float32)
    b_hh = np.asarray(b_hh, np.float32)
    initial_weight = np.asarray(initial_weight, np.float32)
    b_gcn = np.asarray(b_gcn, np.float32)
    assert x.shape == (N_NODES, IN_DIM)

    pre = _preprocess(edge_index)
    dinv, perm, s2 = pre["dinv"], pre["perm"], pre["s2"]

    if "m" not in _cache:
        _cache["m"] = _build_module()
    nc = _cache["m"]

    # dinv-scaled fp16 source table, token layout (partition n%128, rank n//128)
    xsp = np.zeros((NP, IN_DIM), np.float32)
    xsp[:N_NODES] = x * dinv[:, None]
    xs_tiled = np.ascontiguousarray(
        xsp.reshape(RANKS, 128, IN_DIM).transpose(1, 0, 2)
        .reshape(128, RANKS * 128)).astype(np.float16)
    shared = {
        "xs_tiled": xs_tiled,
        "WsumT": np.ascontiguousarray((W_ih + W_hh).T),
        "IW": initial_weight,
        "IWT": np.ascontiguousarray(initial_weight.T),
        "WpT": np.ascontiguousarray(Wp.T),
        "bsum": (b_ih + b_hh).reshape(1, -1),
        "bp_col": np.ascontiguousarray(bp.reshape(-1, 1)),
        "b_gcn": b_gcn.reshape(1, -1),
        "ones_row": np.ones((1, 128), np.float32),
    }
    NLOC = N_NODES // M
    in_maps = []
    for c in range(M):
        pc = perm[c]
        s2p = np.zeros(NPC, np.float32)
        s2p[:NLOC] = s2[pc]
        drip = np.zeros(NPC, np.float32)
        drip[:NLOC] = 1.0 / dinv[pc]
        dlocp = np.zeros(NPC, np.float32)
        dlocp[:NLOC] = dinv[pc]
        in_maps.append({
            **shared,
            "Mt": pre["Mts"][c],
            "s2_row": s2p.reshape(1, -1),
            "dri_row": drip.reshape(1, -1),
            "dinv_col": np.ascontiguousarray(dlocp.reshape(NGRP, 128).T),
        })

    res = run_bass_kernel_spmd(nc, in_maps, list(range(M)))
    LAST_RESULT = res

    out = np.empty((N_NODES, HID), np.float32)
    for c in range(M):
        out[perm[c]] = res.results[c]["out"][:NLOC]
    return out


# revision 12
# speedup vs baseline: 3.2450x; 1.2062x over previous
"""Trainium2 Bass kernel for EvolveGCN-O forward (GCN message passing).

Math (reference):
    h   = x @ Wp + bp
    W   = LSTM-evolved weight from initial_weight (one step, h0=c0=IW)
    hw  = h @ W
    out = D^-1/2 (A+I) D^-1/2 hw + b_gcn

Factored for the kernel:
    out[d] = dinv[d] * (sum_{e: dst=d} dinv[src_e] * x[src_e]) @ (Wp @ W)
             + s2[d]*dinv[d]*(bp @ W) + b_gcn
with s2[d] = sum_{e in(d)} dinv[src_e] (self loops included as edges).

Distribution: nodes (dsts) sharded over 8 NeuronCores (serpentine by degree).
The aggregation over in-edges is computed as a dense blocked matmul: for each
source rank r (128 nodes), xaggT[:, :] += XsT_r @ M_r where Xs is the
dinv-scaled fp16 source-feature table (replicated) and M_r is the fp8 block of
the edge-multiplicity matrix (src-rank r x this core's 1280 dsts; counts are
small integers, exact in fp8). M is streamed from HBM in rank chunks while the
TensorEngine accumulates all 80 ranks into PSUM; no per-edge DMA is needed.
The tiny [H,H] LSTM weight evolution is replicated on every core.
"""

import numpy as np

N_NODES = 10000
N_EDGES = 320000
IN_DIM = 128
HID = 256
M = 8                    # NeuronCores
NP = 10240               # padded node count (mult of 128)
RANKS = NP // 128        # 80
NPC = NP // M            # 1280 padded dsts per core
NGRP = NPC // 128        # 10 dst blocks of 128 per core
RCH = 8                  # ranks per streamed M chunk
NCHK = RANKS // RCH      # 10 chunks

_cache = {}


def _build_module():
    """Build+compile the Bacc module (shapes are static)."""
    import concourse.bacc as bacc
    import concourse.mybir as mybir
    import concourse.tile as tile

    nc = bacc.Bacc("TRN2", target_bir_lowering=False, debug=False,
                   num_devices=M)
    f32, f16, f8 = mybir.dt.float32, mybir.dt.float16, mybir.dt.float8e4

    # ---- DRAM inputs ----
    xs_in = nc.dram_tensor("xs_tiled", [128, RANKS * 128], f16, kind="ExternalInput").ap()
    m_in = nc.dram_tensor("Mt", [128, RANKS * NPC], f8, kind="ExternalInput").ap()
    ws_in = nc.dram_tensor("WsumT", [256, 1024], f32, kind="ExternalInput").ap()
    iw_in = nc.dram_tensor("IW", [256, 256], f32, kind="ExternalInput").ap()
    iwt_in = nc.dram_tensor("IWT", [256, 256], f32, kind="ExternalInput").ap()
    wpt_in = nc.dram_tensor("WpT", [256, 128], f32, kind="ExternalInput").ap()
    bsum_in = nc.dram_tensor("bsum", [1, 1024], f32, kind="ExternalInput").ap()
    bp_in = nc.dram_tensor("bp_col", [256, 1], f32, kind="ExternalInput").ap()
    bgcn_in = nc.dram_tensor("b_gcn", [1, 256], f32, kind="ExternalInput").ap()
    ones_in = nc.dram_tensor("ones_row", [1, 128], f32, kind="ExternalInput").ap()
    s2_in = nc.dram_tensor("s2_row", [1, NPC], f32, kind="ExternalInput").ap()
    dri_in = nc.dram_tensor("dri_row", [1, NPC], f32, kind="ExternalInput").ap()
    dcol_in = nc.dram_tensor("dinv_col", [128, NGRP], f32, kind="ExternalInput").ap()

    out_t = nc.dram_tensor("out", [NPC, HID], f32, kind="ExternalOutput").ap()

    with tile.TileContext(nc) as tc:
        with (
            tc.tile_pool(name="stage", bufs=1) as stpool,
            tc.tile_pool(name="persist", bufs=1) as pp,
            tc.tile_pool(name="mp", bufs=4) as mpool,
            tc.tile_pool(name="op", bufs=2) as opool,
            tc.tile_pool(name="psa", bufs=1, space="PSUM") as psa,
            tc.tile_pool(name="psg", bufs=2, space="PSUM") as psg,
            tc.tile_pool(name="psl", bufs=1, space="PSUM") as psl,
        ):
            # ---------- tiny loads first; PE warmup to raise the clock pstate --
            bsum = pp.tile([1, 1024], f32)
            bgcn = pp.tile([1, 256], f32)
            ones = pp.tile([1, 128], f32)
            nc.sync.dma_start(out=ones[:], in_=ones_in[:])
            nc.sync.dma_start(out=bsum[:], in_=bsum_in[:])
            nc.sync.dma_start(out=bgcn[:], in_=bgcn_in[:])
            wu_ps = psg.tile([32, 64], f32, space="PSUM", tag="ops")
            for _ in range(48):
                nc.tensor.matmul(out=wu_ps[:], lhsT=ones[:, 0:32],
                                 rhs=bsum[:, 0:64], start=True, stop=True)

            wsum = pp.tile([128, 2, 1024], f32)
            iwt = pp.tile([128, 2, 256], f32)
            iw = pp.tile([128, 2, 256], f32)
            wpt = pp.tile([128, 2, 128], f32)
            bp_c = pp.tile([128, 2, 1], f32)
            nc.scalar.dma_start(out=wsum[:], in_=ws_in.rearrange("(k p) c -> p k c", p=128))
            nc.scalar.dma_start(out=iwt[:], in_=iwt_in.rearrange("(k p) c -> p k c", p=128))

            # ---------- stream xs + M per chunk; PE chases ----------
            xs_sb = pp.tile([128, RANKS * 128], f16)
            xagg = pp.tile([128, NPC], f32)
            pa0 = psa.tile([128, 512], f32, space="PSUM", tag="pa0")
            pa1 = psa.tile([128, 512], f32, space="PSUM", tag="pa1")
            pa2 = psa.tile([128, 256], f32, space="PSUM", tag="pa2")
            pa = [pa0, pa1, pa2]
            spans = [(0, 512), (512, 1024), (1024, 1280)]
            w_ev = pp.tile([128, 2, 256], f32)   # evolved GCN weight W
            wpw = pp.tile([128, 256], f32)       # Wp @ W
            bpw = pp.tile([1, 256], f32)         # bp @ W
            s2r = pp.tile([1, NPC], f32)
            drir = pp.tile([1, NPC], f32)
            dcol = pp.tile([128, NGRP], f32)
            Sig = mybir.ActivationFunctionType.Sigmoid
            Tanh = mybir.ActivationFunctionType.Tanh

            def emit_gates(ic):
                # LSTM gates for IW row-chunk ic -> w_ev[:, ic, :]
                for h in range(2):
                    gpsum = psl.tile([128, 512], f32, space="PSUM", tag="gates")
                    gs = slice(512 * h, 512 * (h + 1))
                    nc.tensor.matmul(out=gpsum[:], lhsT=ones[:, :],
                                     rhs=bsum[:, gs], start=True, stop=False)
                    nc.tensor.matmul(out=gpsum[:],
                                     lhsT=iwt[:, 0, 128 * ic:128 * (ic + 1)],
                                     rhs=wsum[:, 0, gs], start=False, stop=False)
                    nc.tensor.matmul(out=gpsum[:],
                                     lhsT=iwt[:, 1, 128 * ic:128 * (ic + 1)],
                                     rhs=wsum[:, 1, gs], start=False, stop=True)
                    a0 = stpool.tile([128, 256], f32, tag=f"a{2*h}")
                    a1 = stpool.tile([128, 256], f32, tag=f"a{2*h+1}")
                    nc.scalar.activation(out=a0[:], in_=gpsum[:, 0:256],
                                         func=(Sig if h == 0 else Tanh))
                    nc.scalar.activation(out=a1[:], in_=gpsum[:, 256:512], func=Sig)
                    if h == 0:
                        si, sf = a0, a1
                    else:
                        tg, so = a0, a1
                c1 = stpool.tile([128, 256], f32, tag="c1")
                nc.vector.tensor_tensor(out=c1[:], in0=sf[:], in1=iw[:, ic, :],
                                        op=mybir.AluOpType.mult)
                c2 = stpool.tile([128, 256], f32, tag="c2")
                nc.vector.tensor_tensor(out=c2[:], in0=si[:], in1=tg[:],
                                        op=mybir.AluOpType.mult)
                cc = stpool.tile([128, 256], f32, tag="cc")
                nc.vector.tensor_tensor(out=cc[:], in0=c1[:], in1=c2[:],
                                        op=mybir.AluOpType.add)
                tcc = stpool.tile([128, 256], f32, tag="tcc")
                nc.scalar.activation(out=tcc[:], in_=cc[:], func=Tanh)
                nc.vector.tensor_tensor(out=w_ev[:, ic, :], in0=so[:], in1=tcc[:],
                                        op=mybir.AluOpType.mult)

            for c in range(NCHK):
                eng = nc.sync if c % 2 == 0 else nc.scalar
                xsl = slice(c * RCH * 128, (c + 1) * RCH * 128)
                eng.dma_start(out=xs_sb[:, xsl], in_=xs_in[:, xsl])
                mt = mpool.tile([128, RCH, NPC], f8, tag="mt")
                eng.dma_start(
                    out=mt[:],
                    in_=m_in[:, c * RCH * NPC:(c + 1) * RCH * NPC]
                        .rearrange("p (r d) -> p r d", d=NPC))
                for k in range(RCH):
                    r = c * RCH + k
                    lhsT = xs_sb[:, r * 128:(r + 1) * 128]
                    for t in range(3):
                        lo, hi = spans[t]
                        nc.tensor.matmul(out=pa[t][:], lhsT=lhsT,
                                         rhs=mt[:, k, lo:hi],
                                         start=(r == 0), stop=(r == RANKS - 1))
                # small-tensor work rides under the stream
                if c == 0:
                    nc.scalar.dma_start(
                        out=iw[:], in_=iw_in.rearrange("(k p) c -> p k c", p=128))
                    nc.scalar.dma_start(
                        out=wpt[:], in_=wpt_in.rearrange("(k p) c -> p k c", p=128))
                    nc.scalar.dma_start(
                        out=bp_c[:], in_=bp_in.rearrange("(k p) c -> p k c", p=128))
                    emit_gates(0)
                elif c == 1:
                    emit_gates(1)
                elif c == 2:
                    wp_ps = psl.tile([128, 256], f32, space="PSUM", tag="gates")
                    nc.tensor.matmul(out=wp_ps[:], lhsT=wpt[:, 0, :],
                                     rhs=w_ev[:, 0, :], start=True, stop=False)
                    nc.tensor.matmul(out=wp_ps[:], lhsT=wpt[:, 1, :],
                                     rhs=w_ev[:, 1, :], start=False, stop=True)
                    nc.vector.tensor_copy(out=wpw[:], in_=wp_ps[:])
                elif c == 3:
                    bp_ps = psl.tile([1, 256], f32, space="PSUM", tag="gates")
                    nc.tensor.matmul(out=bp_ps[:], lhsT=bp_c[:, 0, :],
                                     rhs=w_ev[:, 0, :], start=True, stop=False)
                    nc.tensor.matmul(out=bp_ps[:], lhsT=bp_c[:, 1, :],
                                     rhs=w_ev[:, 1, :], start=False, stop=True)
                    nc.vector.tensor_copy(out=bpw[:], in_=bp_ps[:])
                elif c == 4:
                    nc.sync.dma_start(out=s2r[:], in_=s2_in[:])
                    nc.sync.dma_start(out=drir[:], in_=dri_in[:])
                    nc.sync.dma_start(out=dcol[:], in_=dcol_in[:])

            # ---------- epilogue: out rows = dinv*(xagg@WpW + s2*bpW + dri*bgcn) --
            for t in range(3):
                lo, hi = spans[t]
                nc.scalar.activation(out=xagg[:, lo:hi], in_=pa[t][:],
                                     func=mybir.ActivationFunctionType.Copy)
            for g in range(NGRP):
                ops = psg.tile([128, HID], f32, space="PSUM", tag="ops")
                ds = slice(128 * g, 128 * (g + 1))
                nc.tensor.matmul(out=ops[:], lhsT=s2r[:, ds], rhs=bpw[:],
                                 start=True, stop=False)
                nc.tensor.matmul(out=ops[:], lhsT=drir[:, ds], rhs=bgcn[:],
                                 start=False, stop=False)
                nc.tensor.matmul(out=ops[:], lhsT=xagg[:, ds], rhs=wpw[:],
                                 start=False, stop=True)
                orow = opool.tile([128, HID], f32, tag="orow")
                nc.scalar.activation(out=orow[:], in_=ops[:],
                                     func=mybir.ActivationFunctionType.Copy,
                                     scale=dcol[:, g:g + 1])
                nc.sync.dma_start(
                    out=out_t.rearrange("(g p) h -> g p h", p=128)[g],
                    in_=orow[:],
                )

    nc.compile()
    return nc


def _preprocess(edge_index):
    """Host-side graph preprocessing: degrees, serpentine dst sharding, and
    the per-core fp8 edge-multiplicity matrices."""
    import ml_dtypes

    src = np.asarray(edge_index[0], dtype=np.int64)
    dst = np.asarray(edge_index[1], dtype=np.int64)
    loops = np.arange(N_NODES, dtype=np.int64)
    src_all = np.concatenate([src, loops])
    dst_all = np.concatenate([dst, loops])
    deg = np.bincount(dst_all, minlength=N_NODES).astype(np.float64)
    dinv = (1.0 / np.sqrt(deg)).astype(np.float32)

    # serpentine assignment of degree-sorted nodes to cores
    order = np.argsort(-deg, kind="stable")
    r = np.arange(N_NODES)
    rr = r % (2 * M)
    core_r = np.where(rr < M, rr, 2 * M - 1 - rr)
    lrank_r = (r // (2 * M)) * 2 + (rr >= M)
    core_of = np.empty(N_NODES, np.int64)
    lrank_of = np.empty(N_NODES, np.int64)
    core_of[order] = core_r
    lrank_of[order] = lrank_r

    # per-core permutation: perm[c][l] = global node at local rank l
    perm = np.empty((M, N_NODES // M), np.int64)
    perm[core_of[order], lrank_of[order]] = order

    # per-core fp8 multiplicity matrix Mt[p, r*NPC + d] = #edges (128r+p -> d)
    e_core = core_of[dst_all]
    e_dl = lrank_of[dst_all]
    lut = np.arange(256).astype(ml_dtypes.float8_e4m3)
    Mts = []
    for c in range(M):
        sel = e_core == c
        tok = src_all[sel]
        dl = e_dl[sel]
        mt_u8 = np.zeros((128, RANKS * NPC), np.uint8)
        np.add.at(mt_u8, (tok % 128, (tok // 128) * NPC + dl), 1)
        Mts.append(lut[mt_u8])

    # s2[d] = sum over in-edges of dinv[src] (self loop included)
    s2 = np.bincount(dst_all, weights=dinv[src_all].astype(np.float64),
                     minlength=N_NODES).astype(np.float32)

    return dict(dinv=dinv, perm=perm, Mts=Mts, s2=s2)


LAST_RESULT = None


def kernel(x, edge_index, Wp, bp, W_ih, W_hh, b_ih, b_hh, initial_weight, b_gcn):
    global LAST_RESULT
    from concourse.bass_utils import run_bass_kernel_spmd

    x = np.asarray(x, np.float32)
    Wp = np.asarray(Wp, np.float32)
    bp = np.asarray(bp, np.float32)
    W_ih = np.asarray(W_ih, np.float32)
    W_hh = np.asarray(W_hh, np.float32)
    b_ih = np.asarray(b_ih, np.float32)
    b_hh = np.asarray(b_hh, np.float32)
    initial_weight = np.asarray(initial_weight, np.float32)
    b_gcn = np.asarray(b_gcn, np.float32)
    assert x.shape == (N_NODES, IN_DIM)

    pre = _preprocess(edge_index)
    dinv, perm, s2 = pre["dinv"], pre["perm"], pre["s2"]

    if "m" not in _cache:
        _cache["m"] = _build_module()
    nc = _cache["m"]

    # dinv-scaled fp16 source table, token layout (partition n%128, rank n//128)
    xsp = np.zeros((NP, IN_DIM), np.float32)
    xsp[:N_NODES] = x * dinv[:, None]
    xs_tiled = np.ascontiguousarray(
        xsp.reshape(RANKS, 128, IN_DIM).transpose(1, 0, 2)
        .reshape(128, RANKS * 128)).astype(np.float16)
    shared = {
        "xs_tiled": xs_tiled,
        "WsumT": np.ascontiguousarray((W_ih + W_hh).T),
        "IW": initial_weight,
        "IWT": np.ascontiguousarray(initial_weight.T),
        "WpT": np.ascontiguousarray(Wp.T),
        "bsum": (b_ih + b_hh).reshape(1, -1),
        "bp_col": np.ascontiguousarray(bp.reshape(-1, 1)),
        "b_gcn": b_gcn.reshape(1, -1),
        "ones_row": np.ones((1, 128), np.float32),
    }
    NLOC = N_NODES // M
    in_maps = []
    for c in range(M):
        pc = perm[c]
        s2p = np.zeros(NPC, np.float32)
        s2p[:NLOC] = s2[pc]
        drip = np.zeros(NPC, np.float32)
        drip[:NLOC] = 1.0 / dinv[pc]
        dlocp = np.zeros(NPC, np.float32)
        dlocp[:NLOC] = dinv[pc]
        in_maps.append({
            **shared,
            "Mt": pre["Mts"][c],
            "s2_row": s2p.reshape(1, -1),
            "dri_row": drip.reshape(1, -1),
            "dinv_col": np.ascontiguousarray(dlocp.reshape(NGRP, 128).T),
        })

    res = run_bass_kernel_spmd(nc, in_maps, list(range(M)))
    LAST_RESULT = res

    out = np.empty((N_NODES, HID), np.float32)
    for c in range(M):
        out[perm[c]] = res.results[c]["out"][:NLOC]
    return out


# revision 18
# speedup vs baseline: 3.8165x; 1.1761x over previous
"""Trainium2 Bass kernel for EvolveGCN-O forward (GCN message passing).

Math (reference):
    h   = x @ Wp + bp
    W   = LSTM-evolved weight from initial_weight (one step, h0=c0=IW)
    hw  = h @ W
    out = D^-1/2 (A+I) D^-1/2 hw + b_gcn

Factored for the kernel:
    out[d] = dinv[d] * (sum_{e: dst=d} dinv[src_e] * x[src_e]) @ (Wp @ W)
             + s2[d]*dinv[d]*(bp @ W) + b_gcn
with s2[d] = sum_{e in(d)} dinv[src_e] (self loops included as edges).

Distribution: nodes (dsts) sharded over 8 NeuronCores (serpentine by degree).
The aggregation over in-edges is computed as a dense blocked matmul: for each
source rank r (128 nodes), xaggT[:, :] += XsT_r @ M_r where Xs is the
dinv-scaled fp16 source-feature table (replicated) and M_r is the fp8 block of
the edge-multiplicity matrix (src-rank r x this core's 1280 dsts; counts are
small integers, exact in fp8). M is streamed from HBM in rank chunks while the
TensorEngine accumulates all 80 ranks into PSUM; no per-edge DMA is needed.
The tiny [H,H] LSTM weight evolution is replicated on every core.
"""

import numpy as np

N_NODES = 10000
N_EDGES = 320000
IN_DIM = 128
HID = 256
M = 8                    # NeuronCores
NP = 10240               # padded node count (mult of 128)
RANKS = NP // 128        # 80
NPC = NP // M            # 1280 padded dsts per core
NGRP = NPC // 128        # 10 dst blocks of 128 per core
RCH = 8                  # ranks per streamed M chunk
NCHK = RANKS // RCH      # 10 chunks

_cache = {}


def _build_module():
    """Build+compile the Bacc module (shapes are static)."""
    import concourse.bacc as bacc
    import concourse.mybir as mybir
    import concourse.tile as tile

    nc = bacc.Bacc("TRN2", target_bir_lowering=False, debug=False,
                   num_devices=M)
    f32, f16, f8 = mybir.dt.float32, mybir.dt.float16, mybir.dt.float8e4

    # ---- DRAM inputs ----
    xs_in = nc.dram_tensor("xs_tiled", [128, RANKS * 128], f16, kind="ExternalInput").ap()
    m_in = nc.dram_tensor("Mt", [128, RANKS * NPC], f8, kind="ExternalInput").ap()
    ws_in = nc.dram_tensor("WsumT", [256, 1024], f32, kind="ExternalInput").ap()
    iw_in = nc.dram_tensor("IW", [256, 256], f32, kind="ExternalInput").ap()
    iwt_in = nc.dram_tensor("IWT", [256, 256], f32, kind="ExternalInput").ap()
    wpt_in = nc.dram_tensor("WpT", [256, 128], f32, kind="ExternalInput").ap()
    bsum_in = nc.dram_tensor("bsum", [1, 1024], f32, kind="ExternalInput").ap()
    bp_in = nc.dram_tensor("bp_col", [256, 1], f32, kind="ExternalInput").ap()
    bgcn_in = nc.dram_tensor("b_gcn", [1, 256], f32, kind="ExternalInput").ap()
    ones_in = nc.dram_tensor("ones_row", [1, 128], f32, kind="ExternalInput").ap()
    s2d_in = nc.dram_tensor("s2d_col", [128, NGRP], f32, kind="ExternalInput").ap()
    dcol_in = nc.dram_tensor("dinv_col", [128, NGRP], f32, kind="ExternalInput").ap()

    out_t = nc.dram_tensor("out", [NPC, HID], f32, kind="ExternalOutput").ap()

    with tile.TileContext(nc) as tc:
        with (
            tc.tile_pool(name="stage", bufs=1) as stpool,
            tc.tile_pool(name="persist", bufs=1) as pp,
            tc.tile_pool(name="mp", bufs=4) as mpool,
            tc.tile_pool(name="op", bufs=2) as opool,
            tc.tile_pool(name="psa", bufs=1, space="PSUM") as psa,
            tc.tile_pool(name="psg", bufs=2, space="PSUM") as psg,
            tc.tile_pool(name="psl", bufs=1, space="PSUM") as psl,
        ):
            # ---------- tiny loads first; PE warmup to raise the clock pstate --
            bsum = pp.tile([1, 1024], f32)
            bgcn = pp.tile([1, 256], f32)
            ones = pp.tile([1, 128], f32)
            nc.sync.dma_start(out=ones[:], in_=ones_in[:])
            nc.sync.dma_start(out=bsum[:], in_=bsum_in[:])
            nc.sync.dma_start(out=bgcn[:], in_=bgcn_in[:])

            wsum = pp.tile([128, 2, 1024], f32)
            iwt = pp.tile([128, 2, 256], f32)
            iw = pp.tile([128, 2, 256], f32)
            wpt = pp.tile([128, 2, 128], f32)
            bp_c = pp.tile([128, 2, 1], f32)
            nc.scalar.dma_start(out=wsum[:], in_=ws_in.rearrange("(k p) c -> p k c", p=128))
            nc.scalar.dma_start(out=iwt[:], in_=iwt_in.rearrange("(k p) c -> p k c", p=128))

            # ---------- stream xs + M per chunk; PE chases ----------
            xs_sb = pp.tile([128, RANKS * 128], f16)
            xagg = pp.tile([128, NPC], f32)
            pa0 = psa.tile([128, 512], f32, space="PSUM", tag="pa0")
            pa1 = psa.tile([128, 512], f32, space="PSUM", tag="pa1")
            pa2 = psa.tile([128, 256], f32, space="PSUM", tag="pa2")
            pa = [pa0, pa1, pa2]
            spans = [(0, 512), (512, 1024), (1024, 1280)]
            w_ev = pp.tile([128, 2, 256], f32)   # evolved GCN weight W
            wpw = pp.tile([128, 256], f32)       # Wp @ W
            bpwf = pp.tile([128, 256], f32)      # bp @ W, replicated rows
            t2 = pp.tile([128, NGRP, 256], f32)  # s2*dinv*bpW + b_gcn per block
            s2d = pp.tile([128, NGRP], f32)
            dcol = pp.tile([128, NGRP], f32)
            Sig = mybir.ActivationFunctionType.Sigmoid
            Tanh = mybir.ActivationFunctionType.Tanh

            def emit_gates(ic):
                # LSTM gates for IW row-chunk ic -> w_ev[:, ic, :]
                for h in range(2):
                    gpsum = psl.tile([128, 512], f32, space="PSUM", tag="gates")
                    gs = slice(512 * h, 512 * (h + 1))
                    nc.tensor.matmul(out=gpsum[:], lhsT=ones[:, :],
                                     rhs=bsum[:, gs], start=True, stop=False)
                    nc.tensor.matmul(out=gpsum[:],
                                     lhsT=iwt[:, 0, 128 * ic:128 * (ic + 1)],
                                     rhs=wsum[:, 0, gs], start=False, stop=False)
                    nc.tensor.matmul(out=gpsum[:],
                                     lhsT=iwt[:, 1, 128 * ic:128 * (ic + 1)],
                                     rhs=wsum[:, 1, gs], start=False, stop=True)
                    a0 = stpool.tile([128, 256], f32, tag=f"a{2*h}")
                    a1 = stpool.tile([128, 256], f32, tag=f"a{2*h+1}")
                    nc.scalar.activation(out=a0[:], in_=gpsum[:, 0:256],
                                         func=(Sig if h == 0 else Tanh))
                    nc.scalar.activation(out=a1[:], in_=gpsum[:, 256:512], func=Sig)
                    if h == 0:
                        si, sf = a0, a1
                    else:
                        tg, so = a0, a1
                c1 = stpool.tile([128, 256], f32, tag="c1")
                nc.vector.tensor_tensor(out=c1[:], in0=sf[:], in1=iw[:, ic, :],
                                        op=mybir.AluOpType.mult)
                c2 = stpool.tile([128, 256], f32, tag="c2")
                nc.vector.tensor_tensor(out=c2[:], in0=si[:], in1=tg[:],
                                        op=mybir.AluOpType.mult)
                cc = stpool.tile([128, 256], f32, tag="cc")
                nc.vector.tensor_tensor(out=cc[:], in0=c1[:], in1=c2[:],
                                        op=mybir.AluOpType.add)
                tcc = stpool.tile([128, 256], f32, tag="tcc")
                nc.scalar.activation(out=tcc[:], in_=cc[:], func=Tanh)
                nc.vector.tensor_tensor(out=w_ev[:, ic, :], in0=so[:], in1=tcc[:],
                                        op=mybir.AluOpType.mult)

            for c in range(NCHK):
                eng = nc.sync if c % 2 == 0 else nc.scalar
                xsl = slice(c * RCH * 128, (c + 1) * RCH * 128)
                eng.dma_start(out=xs_sb[:, xsl], in_=xs_in[:, xsl])
                mt = mpool.tile([128, RCH, NPC], f8, tag="mt")
                eng.dma_start(
                    out=mt[:],
                    in_=m_in[:, c * RCH * NPC:(c + 1) * RCH * NPC]
                        .rearrange("p (r d) -> p r d", d=NPC))
                for k in range(RCH):
                    r = c * RCH + k
                    lhsT = xs_sb[:, r * 128:(r + 1) * 128]
                    for t in range(3):
                        lo, hi = spans[t]
                        nc.tensor.matmul(out=pa[t][:], lhsT=lhsT,
                                         rhs=mt[:, k, lo:hi],
                                         start=(r == 0), stop=(r == RANKS - 1))
                # small-tensor work rides under the stream
                if c == 0:
                    nc.scalar.dma_start(
                        out=iw[:], in_=iw_in.rearrange("(k p) c -> p k c", p=128))
                    nc.scalar.dma_start(
                        out=wpt[:], in_=wpt_in.rearrange("(k p) c -> p k c", p=128))
                    nc.scalar.dma_start(
                        out=bp_c[:], in_=bp_in.rearrange("(k p) c -> p k c", p=128))
                    emit_gates(0)
                elif c == 1:
                    emit_gates(1)
                elif c == 2:
                    wp_ps = psl.tile([128, 256], f32, space="PSUM", tag="gates")
                    nc.tensor.matmul(out=wp_ps[:], lhsT=wpt[:, 0, :],
                                     rhs=w_ev[:, 0, :], start=True, stop=False)
                    nc.tensor.matmul(out=wp_ps[:], lhsT=wpt[:, 1, :],
                                     rhs=w_ev[:, 1, :], start=False, stop=True)
                    nc.vector.tensor_copy(out=wpw[:], in_=wp_ps[:])
                elif c == 3:
                    # bp @ W replicated to all 128 partitions:
                    # bpwf = ones128 outer (bp_col.T @ w_ev)
                    bp_ps = psl.tile([1, 256], f32, space="PSUM", tag="gates")
                    nc.tensor.matmul(out=bp_ps[:], lhsT=bp_c[:, 0, :],
                                     rhs=w_ev[:, 0, :], start=True, stop=False)
                    nc.tensor.matmul(out=bp_ps[:], lhsT=bp_c[:, 1, :],
                                     rhs=w_ev[:, 1, :], start=False, stop=True)
                    bpr = stpool.tile([1, 256], f32, tag="bpr")
                    nc.vector.tensor_copy(out=bpr[:], in_=bp_ps[:])
                    bpf_ps = psl.tile([128, 256], f32, space="PSUM", tag="gates2")
                    nc.tensor.matmul(out=bpf_ps[:], lhsT=ones[:, :],
                                     rhs=bpr[:], start=True, stop=True)
                    nc.vector.tensor_copy(out=bpwf[:], in_=bpf_ps[:])
                    nc.sync.dma_start(out=s2d[:], in_=s2d_in[:])
                    nc.sync.dma_start(out=dcol[:], in_=dcol_in[:])
                elif c == 4:
                    # bgcn replicated to all partitions via outer product
                    bgf_ps = psl.tile([128, 256], f32, space="PSUM", tag="gates2")
                    nc.tensor.matmul(out=bgf_ps[:], lhsT=ones[:, :],
                                     rhs=bgcn[:], start=True, stop=True)
                    bgf = stpool.tile([128, 256], f32, tag="bgf")
                    nc.vector.tensor_copy(out=bgf[:], in_=bgf_ps[:])
                    # t2[:, g, :] = s2d[:, g] * bpwf + bgcn  (per dst block)
                    nc.vector.tensor_tensor(
                        out=t2[:],
                        in0=s2d[:].rearrange("p (g o) -> p g o", o=1)
                            .to_broadcast([128, NGRP, 256]),
                        in1=bpwf[:].rearrange("p (o h) -> p o h", o=1)
                            .to_broadcast([128, NGRP, 256]),
                        op=mybir.AluOpType.mult,
                    )
                    nc.vector.tensor_tensor(
                        out=t2[:],
                        in0=t2[:],
                        in1=bgf[:].rearrange("p (o h) -> p o h", o=1)
                            .to_broadcast([128, NGRP, 256]),
                        op=mybir.AluOpType.add,
                    )

            # ---------- epilogue: out rows = dinv*(xagg@WpW + s2*bpW + dri*bgcn) --
            for t in range(3):
                lo, hi = spans[t]
                nc.scalar.activation(out=xagg[:, lo:hi], in_=pa[t][:],
                                     func=mybir.ActivationFunctionType.Copy)
            for g in range(NGRP):
                ops = psg.tile([128, HID], f32, space="PSUM", tag="ops")
                ds = slice(128 * g, 128 * (g + 1))
                nc.tensor.matmul(out=ops[:], lhsT=xagg[:, ds], rhs=wpw[:],
                                 start=True, stop=True)
                opre = opool.tile([128, HID], f32, tag="opre")
                nc.scalar.activation(out=opre[:], in_=ops[:],
                                     func=mybir.ActivationFunctionType.Copy,
                                     scale=dcol[:, g:g + 1])
                orow = opool.tile([128, HID], f32, tag="orow")
                nc.vector.tensor_tensor(out=orow[:], in0=opre[:],
                                        in1=t2[:, g, :],
                                        op=mybir.AluOpType.add)
                nc.sync.dma_start(
                    out=out_t.rearrange("(g p) h -> g p h", p=128)[g],
                    in_=orow[:],
                )

    nc.compile()
    return nc


def _preprocess(edge_index):
    """Host-side graph preprocessing: degrees, serpentine dst sharding, and
    the per-core fp8 edge-multiplicity matrices."""
    import ml_dtypes

    src = np.asarray(edge_index[0], dtype=np.int64)
    dst = np.asarray(edge_index[1], dtype=np.int64)
    loops = np.arange(N_NODES, dtype=np.int64)
    src_all = np.concatenate([src, loops])
    dst_all = np.concatenate([dst, loops])
    deg = np.bincount(dst_all, minlength=N_NODES).astype(np.float64)
    dinv = (1.0 / np.sqrt(deg)).astype(np.float32)

    # serpentine assignment of degree-sorted nodes to cores
    order = np.argsort(-deg, kind="stable")
    r = np.arange(N_NODES)
    rr = r % (2 * M)
    core_r = np.where(rr < M, rr, 2 * M - 1 - rr)
    lrank_r = (r // (2 * M)) * 2 + (rr >= M)
    core_of = np.empty(N_NODES, np.int64)
    lrank_of = np.empty(N_NODES, np.int64)
    core_of[order] = core_r
    lrank_of[order] = lrank_r

    # per-core permutation: perm[c][l] = global node at local rank l
    perm = np.empty((M, N_NODES // M), np.int64)
    perm[core_of[order], lrank_of[order]] = order

    # per-core fp8 multiplicity matrix Mt[p, r*NPC + d] = #edges (128r+p -> d)
    e_core = core_of[dst_all]
    e_dl = lrank_of[dst_all]
    lut = np.arange(256).astype(ml_dtypes.float8_e4m3)
    Mts = []
    for c in range(M):
        sel = e_core == c
        tok = src_all[sel]
        dl = e_dl[sel]
        mt_u8 = np.zeros((128, RANKS * NPC), np.uint8)
        np.add.at(mt_u8, (tok % 128, (tok // 128) * NPC + dl), 1)
        Mts.append(lut[mt_u8])

    # s2[d] = sum over in-edges of dinv[src] (self loop included)
    s2 = np.bincount(dst_all, weights=dinv[src_all].astype(np.float64),
                     minlength=N_NODES).astype(np.float32)

    return dict(dinv=dinv, perm=perm, Mts=Mts, s2=s2)


LAST_RESULT = None


def kernel(x, edge_index, Wp, bp, W_ih, W_hh, b_ih, b_hh, initial_weight, b_gcn):
    global LAST_RESULT
    from concourse.bass_utils import run_bass_kernel_spmd

    x = np.asarray(x, np.float32)
    Wp = np.asarray(Wp, np.float32)
    bp = np.asarray(bp, np.float32)
    W_ih = np.asarray(W_ih, np.float32)
    W_hh = np.asarray(W_hh, np.float32)
    b_ih = np.asarray(b_ih, np.float32)
    b_hh = np.asarray(b_hh, np.float32)
    initial_weight = np.asarray(initial_weight, np.float32)
    b_gcn = np.asarray(b_gcn, np.float32)
    assert x.shape == (N_NODES, IN_DIM)

    pre = _preprocess(edge_index)
    dinv, perm, s2 = pre["dinv"], pre["perm"], pre["s2"]

    if "m" not in _cache:
        _cache["m"] = _build_module()
    nc = _cache["m"]

    # dinv-scaled fp16 source table, token layout (partition n%128, rank n//128)
    xsp = np.zeros((NP, IN_DIM), np.float32)
    xsp[:N_NODES] = x * dinv[:, None]
    xs_tiled = np.ascontiguousarray(
        xsp.reshape(RANKS, 128, IN_DIM).transpose(1, 0, 2)
        .reshape(128, RANKS * 128)).astype(np.float16)
    shared = {
        "xs_tiled": xs_tiled,
        "WsumT": np.ascontiguousarray((W_ih + W_hh).T),
        "IW": initial_weight,
        "IWT": np.ascontiguousarray(initial_weight.T),
        "WpT": np.ascontiguousarray(Wp.T),
        "bsum": (b_ih + b_hh).reshape(1, -1),
        "bp_col": np.ascontiguousarray(bp.reshape(-1, 1)),
        "b_gcn": b_gcn.reshape(1, -1),
        "ones_row": np.ones((1, 128), np.float32),
    }
    NLOC = N_NODES // M
    in_maps = []
    for c in range(M):
        pc = perm[c]
        s2dp = np.zeros(NPC, np.float32)
        s2dp[:NLOC] = s2[pc] * dinv[pc]
        dlocp = np.zeros(NPC, np.float32)
        dlocp[:NLOC] = dinv[pc]
        in_maps.append({
            **shared,
            "Mt": pre["Mts"][c],
            "s2d_col": np.ascontiguousarray(s2dp.reshape(NGRP, 128).T),
            "dinv_col": np.ascontiguousarray(dlocp.reshape(NGRP, 128).T),
        })

    res = run_bass_kernel_spmd(nc, in_maps, list(range(M)))
    LAST_RESULT = res

    out = np.empty((N_NODES, HID), np.float32)
    for c in range(M):
        out[perm[c]] = res.results[c]["out"][:NLOC]
    return out


# revision 25
# speedup vs baseline: 4.1043x; 1.0754x over previous
"""Trainium2 Bass kernel for EvolveGCN-O forward (GCN message passing).

Math (reference):
    h   = x @ Wp + bp
    W   = LSTM-evolved weight from initial_weight (one step, h0=c0=IW)
    hw  = h @ W
    out = D^-1/2 (A+I) D^-1/2 hw + b_gcn

Factored for the kernel:
    out[d] = dinv[d] * (sum_{e: dst=d} dinv[src_e] * x[src_e]) @ (Wp @ W)
             + s2[d]*dinv[d]*(bp @ W) + b_gcn
with s2[d] = sum_{e in(d)} dinv[src_e] (self loops included as edges).

Distribution: nodes (dsts) sharded over 8 NeuronCores (serpentine by degree).
The aggregation over in-edges is computed as a dense blocked matmul: for each
source rank r (128 nodes), xaggT[:, :] += XsT_r @ M_r where Xs is the
dinv-scaled fp16 source-feature table (replicated) and M_r is the fp8 block of
the edge-multiplicity matrix (src-rank r x this core's 1280 dsts; counts are
small integers, exact in fp8). M is streamed from HBM in rank chunks while the
TensorEngine accumulates all 80 ranks into PSUM; no per-edge DMA is needed.
The tiny [H,H] LSTM weight evolution is replicated on every core.
"""

import numpy as np

N_NODES = 10000
N_EDGES = 320000
IN_DIM = 128
HID = 256
M = 8                    # NeuronCores
NP = 10240               # padded node count (mult of 128)
RANKS = NP // 128        # 80
NPC = NP // M            # 1280 padded dsts per core
NGRP = NPC // 128        # 10 dst blocks of 128 per core
RCH = 8                  # ranks per streamed M chunk
NCHK = RANKS // RCH      # 10 chunks

_cache = {}


def _build_module():
    """Build+compile the Bacc module (shapes are static)."""
    import concourse.bacc as bacc
    import concourse.mybir as mybir
    import concourse.tile as tile

    nc = bacc.Bacc("TRN2", target_bir_lowering=False, debug=False,
                   num_devices=M)
    f32, f16, f8 = mybir.dt.float32, mybir.dt.float16, mybir.dt.float8e4
    bf16 = mybir.dt.bfloat16

    # ---- DRAM inputs ----
    xs_in = nc.dram_tensor("xs_tiled", [128, RANKS * 128], f16, kind="ExternalInput").ap()
    m_in = nc.dram_tensor("Mt", [128, RANKS * NPC], f8, kind="ExternalInput").ap()
    ws_in = nc.dram_tensor("WsumT", [256, 1024], bf16, kind="ExternalInput").ap()
    iw_in = nc.dram_tensor("IW", [256, 256], f32, kind="ExternalInput").ap()
    iwt_in = nc.dram_tensor("IWT", [256, 256], bf16, kind="ExternalInput").ap()
    wpt_in = nc.dram_tensor("WpT", [256, 128], f32, kind="ExternalInput").ap()
    bsum_in = nc.dram_tensor("bsum", [1, 1024], f32, kind="ExternalInput").ap()
    bp_in = nc.dram_tensor("bp_col", [256, 1], f32, kind="ExternalInput").ap()
    bgcn_in = nc.dram_tensor("b_gcn", [1, 256], f32, kind="ExternalInput").ap()
    ones_in = nc.dram_tensor("ones_row", [1, 128], f32, kind="ExternalInput").ap()
    s2d_in = nc.dram_tensor("s2d_col", [128, NGRP], f32, kind="ExternalInput").ap()
    dcol_in = nc.dram_tensor("dinv_col", [128, NGRP], f32, kind="ExternalInput").ap()

    out_t = nc.dram_tensor("out", [NPC, HID], f32, kind="ExternalOutput").ap()

    with tile.TileContext(nc) as tc:
        with (
            tc.tile_pool(name="stage", bufs=1) as stpool,
            tc.tile_pool(name="persist", bufs=1) as pp,
            tc.tile_pool(name="mp", bufs=4) as mpool,
            tc.tile_pool(name="op", bufs=2) as opool,
            tc.tile_pool(name="psa", bufs=1, space="PSUM") as psa,
            tc.tile_pool(name="psg", bufs=2, space="PSUM") as psg,
            tc.tile_pool(name="psl", bufs=1, space="PSUM") as psl,
        ):
            # ---------- tiny loads first; PE warmup to raise the clock pstate --
            bsum = pp.tile([1, 1024], f32)
            bgcn = pp.tile([1, 256], f32)
            ones = pp.tile([1, 128], f32)
            # PE warmup on framework consts (no data deps) to kick the clock
            # pstate ramp as early as possible
            cw = nc.const_aps.tensor(1.0, [128, 8])
            cw1 = nc.const_aps.tensor(1.0, [128, 1])
            wu_ps = psg.tile([8, 1], f32, space="PSUM", tag="ops")
            for _ in range(40):
                nc.tensor.matmul(out=wu_ps[:], lhsT=cw, rhs=cw1,
                                 start=True, stop=True)

            nc.sync.dma_start(out=ones[:], in_=ones_in[:])
            nc.sync.dma_start(out=bsum[:], in_=bsum_in[:])
            nc.sync.dma_start(out=bgcn[:], in_=bgcn_in[:])

            wsum = pp.tile([128, 2, 1024], bf16)
            iwt = pp.tile([128, 2, 256], bf16)
            iw = pp.tile([128, 2, 256], f32)
            wpt = pp.tile([128, 2, 128], f32)
            bp_c = pp.tile([128, 2, 1], f32)
            nc.scalar.dma_start(out=wsum[:], in_=ws_in.rearrange("(k p) c -> p k c", p=128))
            nc.scalar.dma_start(out=iwt[:], in_=iwt_in.rearrange("(k p) c -> p k c", p=128))

            # ---------- stream xs + M per chunk; PE chases ----------
            xs_sb = pp.tile([128, RANKS * 128], f16)
            xagg = pp.tile([128, NPC], bf16)
            pa0 = psa.tile([128, 512], f32, space="PSUM", tag="pa0")
            pa1 = psa.tile([128, 512], f32, space="PSUM", tag="pa1")
            pa2 = psa.tile([128, 256], f32, space="PSUM", tag="pa2")
            pa = [pa0, pa1, pa2]
            spans = [(0, 512), (512, 1024), (1024, 1280)]
            w_ev = pp.tile([128, 2, 256], f32)   # evolved GCN weight W
            wpw = pp.tile([128, 256], bf16)      # Wp @ W
            bpwf = pp.tile([128, 256], f32)      # bp @ W, replicated rows
            t2 = pp.tile([128, NGRP, 256], f32)  # s2*dinv*bpW + b_gcn per block
            s2d = pp.tile([128, NGRP], f32)
            dcol = pp.tile([128, NGRP], f32)
            Sig = mybir.ActivationFunctionType.Sigmoid
            Tanh = mybir.ActivationFunctionType.Tanh

            def emit_gates(ic):
                # LSTM gates for IW row-chunk ic -> w_ev[:, ic, :]
                for h in range(2):
                    gpsum = psl.tile([128, 512], f32, space="PSUM", tag="gates")
                    gs = slice(512 * h, 512 * (h + 1))
                    nc.tensor.matmul(out=gpsum[:], lhsT=ones[:, :],
                                     rhs=bsum[:, gs], start=True, stop=False)
                    nc.tensor.matmul(out=gpsum[:],
                                     lhsT=iwt[:, 0, 128 * ic:128 * (ic + 1)],
                                     rhs=wsum[:, 0, gs], start=False, stop=False)
                    nc.tensor.matmul(out=gpsum[:],
                                     lhsT=iwt[:, 1, 128 * ic:128 * (ic + 1)],
                                     rhs=wsum[:, 1, gs], start=False, stop=True)
                    a0 = stpool.tile([128, 256], f32, tag=f"a{2*h}")
                    a1 = stpool.tile([128, 256], f32, tag=f"a{2*h+1}")
                    nc.scalar.activation(out=a0[:], in_=gpsum[:, 0:256],
                                         func=(Sig if h == 0 else Tanh))
                    nc.scalar.activation(out=a1[:], in_=gpsum[:, 256:512], func=Sig)
                    if h == 0:
                        si, sf = a0, a1
                    else:
                        tg, so = a0, a1
                c1 = stpool.tile([128, 256], f32, tag="c1")
                nc.vector.tensor_tensor(out=c1[:], in0=sf[:], in1=iw[:, ic, :],
                                        op=mybir.AluOpType.mult)
                c2 = stpool.tile([128, 256], f32, tag="c2")
                nc.vector.tensor_tensor(out=c2[:], in0=si[:], in1=tg[:],
                                        op=mybir.AluOpType.mult)
                cc = stpool.tile([128, 256], f32, tag="cc")
                nc.vector.tensor_tensor(out=cc[:], in0=c1[:], in1=c2[:],
                                        op=mybir.AluOpType.add)
                tcc = stpool.tile([128, 256], f32, tag="tcc")
                nc.scalar.activation(out=tcc[:], in_=cc[:], func=Tanh)
                nc.vector.tensor_tensor(out=w_ev[:, ic, :], in0=so[:], in1=tcc[:],
                                        op=mybir.AluOpType.mult)

            nc.scalar.dma_start(
                out=iw[:], in_=iw_in.rearrange("(k p) c -> p k c", p=128))
            nc.scalar.dma_start(
                out=wpt[:], in_=wpt_in.rearrange("(k p) c -> p k c", p=128))
            nc.scalar.dma_start(
                out=bp_c[:], in_=bp_in.rearrange("(k p) c -> p k c", p=128))
            emit_gates(0)
            emit_gates(1)

            for c in range(NCHK):
                eng = nc.sync if c % 2 == 0 else nc.scalar
                xsl = slice(c * RCH * 128, (c + 1) * RCH * 128)
                eng.dma_start(out=xs_sb[:, xsl], in_=xs_in[:, xsl])
                mt = mpool.tile([128, RCH, NPC], f8, tag="mt")
                eng.dma_start(
                    out=mt[:],
                    in_=m_in[:, c * RCH * NPC:(c + 1) * RCH * NPC]
                        .rearrange("p (r d) -> p r d", d=NPC))
                for k in range(RCH):
                    r = c * RCH + k
                    lhsT = xs_sb[:, r * 128:(r + 1) * 128]
                    for t in range(3):
                        lo, hi = spans[t]
                        nc.tensor.matmul(out=pa[t][:], lhsT=lhsT,
                                         rhs=mt[:, k, lo:hi],
                                         start=(r == 0), stop=(r == RANKS - 1))
                # small-tensor work rides under the stream
                if c == 0:
                    wp_ps = psl.tile([128, 256], f32, space="PSUM", tag="gates")
                    nc.tensor.matmul(out=wp_ps[:], lhsT=wpt[:, 0, :],
                                     rhs=w_ev[:, 0, :], start=True, stop=False)
                    nc.tensor.matmul(out=wp_ps[:], lhsT=wpt[:, 1, :],
                                     rhs=w_ev[:, 1, :], start=False, stop=True)
                    nc.vector.tensor_copy(out=wpw[:], in_=wp_ps[:])
                elif c == 1:
                    # bp @ W replicated to all 128 partitions:
                    # bpwf = ones128 outer (bp_col.T @ w_ev)
                    bp_ps = psl.tile([1, 256], f32, space="PSUM", tag="gates")
                    nc.tensor.matmul(out=bp_ps[:], lhsT=bp_c[:, 0, :],
                                     rhs=w_ev[:, 0, :], start=True, stop=False)
                    nc.tensor.matmul(out=bp_ps[:], lhsT=bp_c[:, 1, :],
                                     rhs=w_ev[:, 1, :], start=False, stop=True)
                    bpr = stpool.tile([1, 256], f32, tag="bpr")
                    nc.vector.tensor_copy(out=bpr[:], in_=bp_ps[:])
                    bpf_ps = psl.tile([128, 256], f32, space="PSUM", tag="gates2")
                    nc.tensor.matmul(out=bpf_ps[:], lhsT=ones[:, :],
                                     rhs=bpr[:], start=True, stop=True)
                    nc.vector.tensor_copy(out=bpwf[:], in_=bpf_ps[:])
                    nc.sync.dma_start(out=s2d[:], in_=s2d_in[:])
                    nc.sync.dma_start(out=dcol[:], in_=dcol_in[:])
                elif c == 2:
                    # bgcn replicated to all partitions via outer product
                    bgf_ps = psl.tile([128, 256], f32, space="PSUM", tag="gates2")
                    nc.tensor.matmul(out=bgf_ps[:], lhsT=ones[:, :],
                                     rhs=bgcn[:], start=True, stop=True)
                    bgf = stpool.tile([128, 256], f32, tag="bgf")
                    nc.vector.tensor_copy(out=bgf[:], in_=bgf_ps[:])
                    # t2[:, g, :] = s2d[:, g] * bpwf + bgcn  (per dst block)
                    nc.vector.tensor_tensor(
                        out=t2[:],
                        in0=s2d[:].rearrange("p (g o) -> p g o", o=1)
                            .to_broadcast([128, NGRP, 256]),
                        in1=bpwf[:].rearrange("p (o h) -> p o h", o=1)
                            .to_broadcast([128, NGRP, 256]),
                        op=mybir.AluOpType.mult,
                    )
                    nc.vector.tensor_tensor(
                        out=t2[:],
                        in0=t2[:],
                        in1=bgf[:].rearrange("p (o h) -> p o h", o=1)
                            .to_broadcast([128, NGRP, 256]),
                        op=mybir.AluOpType.add,
                    )

            # ---------- epilogue: out rows = dinv*(xagg@WpW + s2*bpW + dri*bgcn) --
            for t in range(3):
                lo, hi = spans[t]
                nc.scalar.activation(out=xagg[:, lo:hi], in_=pa[t][:],
                                     func=mybir.ActivationFunctionType.Copy)
            for g in range(NGRP):
                ops = psg.tile([128, HID], f32, space="PSUM", tag="ops")
                ds = slice(128 * g, 128 * (g + 1))
                nc.tensor.matmul(out=ops[:], lhsT=xagg[:, ds], rhs=wpw[:],
                                 start=True, stop=True)
                opre = opool.tile([128, HID], f32, tag="opre")
                nc.scalar.activation(out=opre[:], in_=ops[:],
                                     func=mybir.ActivationFunctionType.Copy,
                                     scale=dcol[:, g:g + 1])
                orow = opool.tile([128, HID], f32, tag="orow")
                nc.vector.tensor_tensor(out=orow[:], in0=opre[:],
                                        in1=t2[:, g, :],
                                        op=mybir.AluOpType.add)
                nc.sync.dma_start(
                    out=out_t.rearrange("(g p) h -> g p h", p=128)[g],
                    in_=orow[:],
                )

    nc.compile()
    return nc


def _preprocess(edge_index):
    """Host-side graph preprocessing: degrees, serpentine dst sharding, and
    the per-core fp8 edge-multiplicity matrices."""
    import ml_dtypes

    src = np.asarray(edge_index[0], dtype=np.int64)
    dst = np.asarray(edge_index[1], dtype=np.int64)
    loops = np.arange(N_NODES, dtype=np.int64)
    src_all = np.concatenate([src, loops])
    dst_all = np.concatenate([dst, loops])
    deg = np.bincount(dst_all, minlength=N_NODES).astype(np.float64)
    dinv = (1.0 / np.sqrt(deg)).astype(np.float32)

    # serpentine assignment of degree-sorted nodes to cores
    order = np.argsort(-deg, kind="stable")
    r = np.arange(N_NODES)
    rr = r % (2 * M)
    core_r = np.where(rr < M, rr, 2 * M - 1 - rr)
    lrank_r = (r // (2 * M)) * 2 + (rr >= M)
    core_of = np.empty(N_NODES, np.int64)
    lrank_of = np.empty(N_NODES, np.int64)
    core_of[order] = core_r
    lrank_of[order] = lrank_r

    # per-core permutation: perm[c][l] = global node at local rank l
    perm = np.empty((M, N_NODES // M), np.int64)
    perm[core_of[order], lrank_of[order]] = order

    # per-core fp8 multiplicity matrix Mt[p, r*NPC + d] = #edges (128r+p -> d)
    e_core = core_of[dst_all]
    e_dl = lrank_of[dst_all]
    lut = np.arange(256).astype(ml_dtypes.float8_e4m3)
    Mts = []
    for c in range(M):
        sel = e_core == c
        tok = src_all[sel]
        dl = e_dl[sel]
        mt_u8 = np.zeros((128, RANKS * NPC), np.uint8)
        np.add.at(mt_u8, (tok % 128, (tok // 128) * NPC + dl), 1)
        Mts.append(lut[mt_u8])

    # s2[d] = sum over in-edges of dinv[src] (self loop included)
    s2 = np.bincount(dst_all, weights=dinv[src_all].astype(np.float64),
                     minlength=N_NODES).astype(np.float32)

    return dict(dinv=dinv, perm=perm, Mts=Mts, s2=s2)


LAST_RESULT = None


def kernel(x, edge_index, Wp, bp, W_ih, W_hh, b_ih, b_hh, initial_weight, b_gcn):
    global LAST_RESULT
    from concourse.bass_utils import run_bass_kernel_spmd

    x = np.asarray(x, np.float32)
    Wp = np.asarray(Wp, np.float32)
    bp = np.asarray(bp, np.float32)
    W_ih = np.asarray(W_ih, np.float32)
    W_hh = np.asarray(W_hh, np.float32)
    b_ih = np.asarray(b_ih, np.float32)
    b_hh = np.asarray(b_hh, np.float32)
    initial_weight = np.asarray(initial_weight, np.float32)
    b_gcn = np.asarray(b_gcn, np.float32)
    assert x.shape == (N_NODES, IN_DIM)

    pre = _preprocess(edge_index)
    dinv, perm, s2 = pre["dinv"], pre["perm"], pre["s2"]

    if "m" not in _cache:
        _cache["m"] = _build_module()
    nc = _cache["m"]

    # dinv-scaled fp16 source table, token layout (partition n%128, rank n//128)
    xsp = np.zeros((NP, IN_DIM), np.float32)
    xsp[:N_NODES] = x * dinv[:, None]
    xs_tiled = np.ascontiguousarray(
        xsp.reshape(RANKS, 128, IN_DIM).transpose(1, 0, 2)
        .reshape(128, RANKS * 128)).astype(np.float16)
    import ml_dtypes
    shared = {
        "xs_tiled": xs_tiled,
        "WsumT": np.ascontiguousarray((W_ih + W_hh).T).astype(ml_dtypes.bfloat16),
        "IW": initial_weight,
        "IWT": np.ascontiguousarray(initial_weight.T).astype(ml_dtypes.bfloat16),
        "WpT": np.ascontiguousarray(Wp.T),
        "bsum": (b_ih + b_hh).reshape(1, -1),
        "bp_col": np.ascontiguousarray(bp.reshape(-1, 1)),
        "b_gcn": b_gcn.reshape(1, -1),
        "ones_row": np.ones((1, 128), np.float32),
    }
    NLOC = N_NODES // M
    in_maps = []
    for c in range(M):
        pc = perm[c]
        s2dp = np.zeros(NPC, np.float32)
        s2dp[:NLOC] = s2[pc] * dinv[pc]
        dlocp = np.zeros(NPC, np.float32)
        dlocp[:NLOC] = dinv[pc]
        in_maps.append({
            **shared,
            "Mt": pre["Mts"][c],
            "s2d_col": np.ascontiguousarray(s2dp.reshape(NGRP, 128).T),
            "dinv_col": np.ascontiguousarray(dlocp.reshape(NGRP, 128).T),
        })

    res = run_bass_kernel_spmd(nc, in_maps, list(range(M)))
    LAST_RESULT = res

    out = np.empty((N_NODES, HID), np.float32)
    for c in range(M):
        out[perm[c]] = res.results[c]["out"][:NLOC]
    return out
